# revision 1
# baseline (speedup 1.0000x reference)
"""Trainium2 Bass kernel for nn_Encoder (dense transformer block), 8 NeuronCores.

Strategy (single chip, 8 cores):
  Phase A (head-parallel): core c computes attention for heads {2c, 2c+1}.
    All activations are kept "transposed" (feature dim on SBUF partitions) so
    every matmul consumes naturally-laid-out operands and no fp32 transposes
    are ever needed on device; the host pre-transposes X and all weights.
    softmax(relu(s)) is computed as p = max(exp(s/8), 1) (exp is monotonic),
    and the row sums come for free as a 65th column of the p @ [V | 1] matmul.
  Host gathers per-head ctx blocks (2 MiB/core) between launches.
  Phase B (row-parallel): core c takes 512 of the 4096 token rows:
    ctx @ Wo.T (+X residual), LN1, FFN (ReLU), LN2. LayerNorm reductions run
    over the partition dim via tiny ones-vector matmuls on the PE.

kernel() is self-contained: it compiles both phase programs on first call
(cached in module globals) and runs them via run_bass_kernel_spmd.
"""

import os
import sys

for _p in ("/opt/trn_rl_repo",):
    if _p not in sys.path:
        sys.path.insert(0, _p)

# The Bass SPMD path executes through jax/PJRT on the axon platform; make
# sure a caller-pinned JAX_PLATFORMS=cpu doesn't hide the NeuronCores.
_jp = os.environ.get("JAX_PLATFORMS")
if _jp is not None and "axon" not in _jp:
    os.environ["JAX_PLATFORMS"] = "axon," + _jp

import numpy as np

import concourse.bass as bass
import concourse.mybir as mybir
import concourse.tile as tile
from concourse import bacc
from concourse.bass_utils import run_bass_kernel_spmd

F32 = mybir.dt.float32
F32R = mybir.dt.float32r
AF = mybir.ActivationFunctionType
OP = mybir.AluOpType


def _mm(nc, out, lhsT, rhs, **kw):
    # fp32r: 1-pass FP22 matmul (4x faster than 4-pass true-fp32 for N>=256)
    nc.tensor.matmul(out, lhsT.bitcast(F32R), rhs.bitcast(F32R), **kw)

N_CORES = 8
B, S, D, H, DH, FF = 2, 2048, 1024, 16, 64, 4096
N = B * S            # 4096 token rows
P = 128
QC = N // N_CORES    # 512 rows per core in phase B
HPC = H // N_CORES   # 2 heads per core in phase A
EPS = 1e-5

_CACHE = {}


# --------------------------------------------------------------------------
# Phase A: per-core head-parallel attention.
# Inputs (per core):
#   xt  [D, N]     X^T (full, replicated)
#   wqt [D, 128]   Wq^T columns for this core's two heads
#   wkt [D, 128]   Wk^T columns
#   wvt [D, 128]   Wo^T columns (value projection uses W_o in this model)
# Output:
#   ctx_t [128, N] softmax(relu(qk/8)) @ v, transposed; rows = the two heads'
#                  64-dim blocks stacked, cols = (b, s) token index.
# --------------------------------------------------------------------------
def _build_phase_a():
    nc = bacc.Bacc("TRN2", target_bir_lowering=False, debug=False,
                   num_devices=N_CORES)
    xt = nc.dram_tensor("xt", [D, N], F32R, kind="ExternalInput")
    wqt = nc.dram_tensor("wqt", [D, P], F32R, kind="ExternalInput")
    wkt = nc.dram_tensor("wkt", [D, P], F32R, kind="ExternalInput")
    wvt = nc.dram_tensor("wvt", [D, P], F32R, kind="ExternalInput")
    idm = nc.dram_tensor("idm", [P, DH], F32R, kind="ExternalInput")
    ctx_t = nc.dram_tensor("ctx_t", [P, N], F32, kind="ExternalOutput")

    KD = D // P        # 8 contraction chunks over D
    NQ = N // 512      # 8 qi chunks of 512 over all tokens
    KI = S // P        # 16 ki chunks of 128 per batch

    with tile.TileContext(nc) as tc:
        with tc.tile_pool(name="persist", bufs=1) as persist:
            # Persistent SBUF, split per batch so batch-1 projection writes
            # never serialize against batch-0 attention reads: projected Q^T,
            # K^T, V^T (1 MiB each per batch) and V' (natural layout per
            # ki-chunk: [v_h0(64) | 1 | v_h1(64) | 1]).
            qt_sb = [persist.tile([P, S], F32R, name=f"qt{b_}") for b_ in range(B)]
            kt_sb = [persist.tile([P, S], F32R, name=f"kt{b_}") for b_ in range(B)]
            vt_sb = [persist.tile([P, S], F32R, name=f"vt{b_}") for b_ in range(B)]
            vp_sb = [persist.tile([P, KI, 2 * (DH + 1)], F32R, name=f"vp{b_}")
                     for b_ in range(B)]
            wq_sb = persist.tile([P, KD, P], F32R)
            wk_sb = persist.tile([P, KD, P], F32R)
            wv_sb = persist.tile([P, KD, P], F32R)
            id_sb = persist.tile([P, DH], F32R)

            nc.sync.dma_start(wq_sb[:], wqt.ap().rearrange("(kc p) m -> p kc m", p=P))
            nc.sync.dma_start(wk_sb[:], wkt.ap().rearrange("(kc p) m -> p kc m", p=P))
            nc.sync.dma_start(wv_sb[:], wvt.ap().rearrange("(kc p) m -> p kc m", p=P))
            nc.sync.dma_start(id_sb[:], idm.ap())
            for b_ in range(B):
                # ones columns of V' (columns DH and 2*DH+1)
                nc.vector.memset(vp_sb[b_][:, :, DH:DH + 1].bitcast(F32), 1.0)
                nc.vector.memset(
                    vp_sb[b_][:, :, 2 * DH + 1:2 * DH + 2].bitcast(F32), 1.0)

            # ---------------- fused projections + attention ----------------
            # Projections run in t-layout (N=512 keeps fp32r at 1 cyc/row); V
            # is PE-transposed into natural layout for the ctx matmul. Batch
            # 0's projections form the prologue; batch 1's are interleaved
            # into batch 0's attention chunks to fill the PE slack while the
            # ScalarE exp pass (the bottleneck) runs. The attention itself is
            # software-pipelined at ki-chunk granularity: chunk i+1's score
            # matmuls interleave with chunk i's ctx matmuls.
            with (
                tc.tile_pool(name="xa", bufs=9) as xpool,
                tc.tile_pool(name="accp", bufs=2, space="PSUM") as accp,
                tc.tile_pool(name="slabp", bufs=19) as slabp,
                tc.tile_pool(name="smallp", bufs=2) as smallp,
                tc.tile_pool(name="coutp", bufs=2) as coutp,
                tc.tile_pool(name="pss", bufs=2, space="PSUM") as pss,
                tc.tile_pool(name="psc", bufs=1, space="PSUM") as psc,
            ):

                def proj_chunk(b_, o):
                    """Project one 512-token slice of batch b_ into qt/kt/vt.

                    Three sequential PSUM accumulation chains over a shared
                    single-slot pool tag keep the PSUM footprint at 2 banks.
                    """
                    tiles = []
                    for kc in range(KD):
                        xt_tile = xpool.tile([P, 512], F32R, name="xt_tile")
                        nc.sync.dma_start(
                            xt_tile[:],
                            xt[kc * P:(kc + 1) * P,
                               b_ * S + o * 512:b_ * S + (o + 1) * 512])
                        tiles.append(xt_tile)
                    for w_sb, dst in ((wq_sb, qt_sb[b_]), (wk_sb, kt_sb[b_]),
                                      (wv_sb, vt_sb[b_])):
                        acc = accp.tile([P, 512], F32, name="acc_ps")
                        for kc in range(KD):
                            _mm(nc, acc[:], w_sb[:, kc], tiles[kc][:],
                                start=(kc == 0), stop=(kc == KD - 1))
                        nc.vector.tensor_copy(
                            dst[:, o * 512:(o + 1) * 512], acc[:])

                def transp_chunk(b_, kc2):
                    """PE-transpose one [64,128] V^T block per head into V'."""
                    for hh in range(2):
                        tp = accp.tile([P, DH], F32R, name="acc_ps")
                        nc.tensor.transpose(
                            tp[:, :DH],
                            vt_sb[b_][hh * DH:(hh + 1) * DH,
                                      kc2 * P:(kc2 + 1) * P],
                            id_sb[hh * DH:(hh + 1) * DH, :])
                        nc.vector.tensor_copy(
                            vp_sb[b_][:, kc2,
                                      hh * (DH + 1):hh * (DH + 1) + DH],
                            tp[:, :DH])
                chunks = [(b_, o) for b_ in range(B) for o in range(S // 512)]
                state = {}

                def emit_scores(idx, kc):
                    b_, o = chunks[idx]
                    qs = slice(o * 512, (o + 1) * 512)
                    ks = slice(kc * P, (kc + 1) * P)
                    s_ps = pss.tile([P, 1024], F32, name="s_ps")
                    _mm(nc, s_ps[:, 0:512], kt_sb[b_][0:DH, ks],
                        qt_sb[b_][0:DH, qs], start=True, stop=True)
                    _mm(nc, s_ps[:, 512:1024], kt_sb[b_][DH:2 * DH, ks],
                        qt_sb[b_][DH:2 * DH, qs], start=True, stop=True)
                    slab = slabp.tile([P, 1024], F32R, name="slab")
                    nc.scalar.activation(slab[:], s_ps[:], AF.Exp, scale=0.125)
                    nc.vector.tensor_scalar_max(slab[:], slab[:], 1.0)
                    state[idx]["slabs"].append(slab)

                def emit_ctx(idx, kc):
                    b_, o = chunks[idx]
                    st_, sp_ = kc == 0, kc == KI - 1
                    c0, c1 = state[idx]["c0"], state[idx]["c1"]
                    slab = state[idx]["slabs"][kc]
                    _mm(nc, c0[:], vp_sb[b_][:, kc, 0:DH + 1], slab[:, 0:512],
                        start=st_, stop=sp_)
                    _mm(nc, c1[:], vp_sb[b_][:, kc, DH + 1:2 * DH + 2],
                        slab[:, 512:1024], start=st_, stop=sp_)

                def emit_normalize(idx):
                    b_, o = chunks[idx]
                    qs = slice(b_ * S + o * 512, b_ * S + (o + 1) * 512)
                    c0, c1 = state[idx]["c0"], state[idx]["c1"]
                    inv0 = smallp.tile([1, 512], F32, name="inv0")
                    inv1 = smallp.tile([1, 512], F32, name="inv1")
                    nc.vector.reciprocal(inv0[:], c0[DH:DH + 1, :])
                    nc.vector.reciprocal(inv1[:], c1[DH:DH + 1, :])
                    inv0b = smallp.tile([DH, 512], F32, name="inv0b")
                    inv1b = smallp.tile([DH, 512], F32, name="inv1b")
                    nc.gpsimd.partition_broadcast(inv0b[:], inv0[:])
                    nc.gpsimd.partition_broadcast(inv1b[:], inv1[:])
                    cout0 = coutp.tile([DH, 512], F32, name="cout0")
                    cout1 = coutp.tile([DH, 512], F32, name="cout1")
                    nc.vector.tensor_mul(cout0[:], c0[0:DH, :], inv0b[:])
                    nc.vector.tensor_mul(cout1[:], c1[0:DH, :], inv1b[:])
                    nc.sync.dma_start(ctx_t[0:DH, qs], cout0[:])
                    nc.sync.dma_start(ctx_t[DH:2 * DH, qs], cout1[:])
                    del state[idx]

                NO = S // 512   # 4 proj chunks per batch
                TPO = KI // NO  # 4 transposes per proj chunk
                # prologue: batch-0 projections with chunk-0 score matmuls
                # folded in per o-slice, so the ScalarE exp pass starts after
                # the first projection chunk (~7 us) instead of after all of
                # batch 0 (~28 us).
                state[0] = {
                    "c0": psc.tile([DH + 1, 512], F32, name="c0"),
                    "c1": psc.tile([DH + 1, 512], F32, name="c1"),
                    "slabs": [],
                }
                for o in range(NO):
                    proj_chunk(0, o)
                    for t in range(TPO):
                        transp_chunk(0, o * TPO + t)
                    for kc in range(o * TPO, (o + 1) * TPO):
                        emit_scores(0, kc)
                # attention, with batch-1 projections/transposes interleaved
                # into batch-0's chunks (idx 1..4)
                for idx in range(1, len(chunks)):
                    state[idx] = {
                        "c0": psc.tile([DH + 1, 512], F32, name="c0"),
                        "c1": psc.tile([DH + 1, 512], F32, name="c1"),
                        "slabs": [],
                    }
                    if idx <= NO:
                        proj_chunk(1, idx - 1)
                        for t in range(TPO):
                            transp_chunk(1, (idx - 1) * TPO + t)
                    for kc in range(KI):
                        emit_scores(idx, kc)
                        emit_ctx(idx - 1, kc)
                    emit_normalize(idx - 1)
                last = len(chunks) - 1
                for kc in range(KI):
                    emit_ctx(last, kc)
                emit_normalize(last)
    nc.compile()
    return nc


# --------------------------------------------------------------------------
# Phase B: per-core row-parallel Wo-proj + AddNorm1 + FFN + AddNorm2.
# Inputs (per core, qi = this core's 512 token rows):
#   ct  [D, QC]    ctx^T slice
#   xts [D, QC]    X^T slice (residual 1)
#   wot [D, D]     Wo^T
#   w1t [D, FF]    W1^T
#   w2t [FF, D]    W2^T
#   g1,be1,g2,be2 [P, D//P]  ln params, feature-on-partition layout
#   b1t [P, FF//P], b2t [P, D//P]
# Output: out_t [D, QC]
# --------------------------------------------------------------------------
def _build_phase_b():
    nc = bacc.Bacc("TRN2", target_bir_lowering=False, debug=False,
                   num_devices=N_CORES)
    ct = nc.dram_tensor("ct", [D, QC], F32R, kind="ExternalInput")
    xts = nc.dram_tensor("xts", [D, QC], F32, kind="ExternalInput")
    wot = nc.dram_tensor("wot", [D, D], F32R, kind="ExternalInput")
    w1t = nc.dram_tensor("w1t", [D, FF], F32R, kind="ExternalInput")
    w2t = nc.dram_tensor("w2t", [FF, D], F32R, kind="ExternalInput")
    g1 = nc.dram_tensor("g1", [P, D // P], F32, kind="ExternalInput")
    be1 = nc.dram_tensor("be1", [P, D // P], F32, kind="ExternalInput")
    g2 = nc.dram_tensor("g2", [P, D // P], F32, kind="ExternalInput")
    be2 = nc.dram_tensor("be2", [P, D // P], F32, kind="ExternalInput")
    b1t = nc.dram_tensor("b1t", [P, FF // P], F32, kind="ExternalInput")
    b2t = nc.dram_tensor("b2t", [P, D // P], F32, kind="ExternalInput")
    out_t = nc.dram_tensor("out_t", [D, QC], F32, kind="ExternalOutput")

    KD = D // P     # 8
    KF = FF // P    # 32

    def layernorm(nc, tc, pools, y_sb, g_sb, be_sb, z_sb, ones, tag):
        """t-layout layernorm: y_sb [P, KD, 512] -> z_sb (may alias layout)."""
        smallp, sqp, bcp = pools
        import contextlib
        ctx = contextlib.ExitStack()
        psst = ctx.enter_context(
            tc.tile_pool(name=f"psst_{tag}", bufs=1, space="PSUM"))
        st_ps = psst.tile([1, 1024], F32, name="st_ps")
        for kc in range(KD):
            _mm(nc, st_ps[:, 0:512], ones[:], y_sb[:, kc],
                             start=(kc == 0), stop=(kc == KD - 1))
        for kc in range(KD):
            sq = sqp.tile([P, 512], F32R, name="sq")
            nc.vector.tensor_mul(sq[:], y_sb[:, kc], y_sb[:, kc])
            _mm(nc, st_ps[:, 512:1024], ones[:], sq[:],
                             start=(kc == 0), stop=(kc == KD - 1))
        stats = smallp.tile([1, 1024], F32, name="stats")
        nc.vector.tensor_scalar(out=stats[:], in0=st_ps[:], scalar1=1.0 / D,
                                scalar2=None, op0=OP.mult)
        mean = stats[:, 0:512]
        ex2 = stats[:, 512:1024]
        msq = smallp.tile([1, 512], F32, name="msq")
        nc.vector.tensor_mul(msq[:], mean, mean)
        var = smallp.tile([1, 512], F32, name="var")
        nc.vector.tensor_sub(var[:], ex2, msq[:])
        nc.vector.tensor_scalar_add(var[:], var[:], EPS)
        std = smallp.tile([1, 512], F32, name="std")
        nc.scalar.activation(std[:], var[:], AF.Sqrt)
        rstd = smallp.tile([1, 512], F32, name="rstd")
        nc.vector.reciprocal(rstd[:], std[:])
        ms = smallp.tile([1, 512], F32, name="ms")
        nc.vector.tensor_mul(ms[:], mean, rstd[:])
        rstd_b = bcp.tile([P, 512], F32, name="rstd_b")
        ms_b = bcp.tile([P, 512], F32, name="ms_b")
        nc.gpsimd.partition_broadcast(rstd_b[:], rstd[:])
        nc.gpsimd.partition_broadcast(ms_b[:], ms[:])
        for kc in range(KD):
            t = sqp.tile([P, 512], F32, name="t_ln")
            nc.vector.tensor_mul(t[:], y_sb[:, kc], rstd_b[:])
            nc.vector.tensor_sub(t[:], t[:], ms_b[:])
            nc.vector.tensor_scalar(out=z_sb[:, kc], in0=t[:],
                                    scalar1=g_sb[:, kc:kc + 1],
                                    scalar2=be_sb[:, kc:kc + 1],
                                    op0=OP.mult, op1=OP.add)
        ctx.close()

    with tile.TileContext(nc) as tc:
        with (
            tc.tile_pool(name="persist", bufs=1) as persist,
            tc.tile_pool(name="wp", bufs=6) as wp,
            tc.tile_pool(name="sqp", bufs=3) as sqp,
            tc.tile_pool(name="smallp", bufs=1) as smallp,
            tc.tile_pool(name="bcp", bufs=2) as bcp,
        ):
            ct_sb = persist.tile([P, KD, QC], F32R)
            xts_sb = persist.tile([P, KD, QC], F32)
            y1_sb = persist.tile([P, KD, QC], F32R)
            z1_sb = persist.tile([P, KD, QC], F32R)
            h_sb = persist.tile([P, KF, QC], F32R)
            # y2 reuses y1's slot (y1 dead after LN1); z2 reuses ct's (dead
            # after B1). Tag sharing makes Tile serialize via WAR edges.
            y2_sb = persist.tile([P, KD, QC], F32R, tag="y1_sb")
            z2_sb = persist.tile([P, KD, QC], F32, tag="ct_sb")
            g1_sb = persist.tile([P, KD], F32)
            be1_sb = persist.tile([P, KD], F32)
            g2_sb = persist.tile([P, KD], F32)
            be2_sb = persist.tile([P, KD], F32)
            b1t_sb = persist.tile([P, KF], F32)
            b2t_sb = persist.tile([P, KD], F32)
            ones = persist.tile([P, 1], F32R)

            nc.sync.dma_start(ct_sb[:], ct.ap().rearrange("(kc p) q -> p kc q", p=P))
            nc.sync.dma_start(xts_sb[:], xts.ap().rearrange("(kc p) q -> p kc q", p=P))
            for t_sb, t_dr in ((g1_sb, g1), (be1_sb, be1), (g2_sb, g2),
                               (be2_sb, be2), (b1t_sb, b1t), (b2t_sb, b2t)):
                nc.sync.dma_start(t_sb[:], t_dr.ap())
            nc.vector.memset(ones[:].bitcast(F32), 1.0)

            # ---- B1: att_out = Wo @ ct (+ X residual) ----
            with tc.tile_pool(name="psa", bufs=1, space="PSUM") as psa:
                for mg in range(2):
                    a_ps = [psa.tile([P, 512], F32, name=f"mm_ps{i}")
                            for i in range(4)]
                    for kc in range(KD):
                        w_tile = wp.tile([P, 512], F32R, name="wo_tile")
                        nc.sync.dma_start(
                            w_tile[:],
                            wot[kc * P:(kc + 1) * P, mg * 512:(mg + 1) * 512])
                        for i in range(4):
                            _mm(nc, a_ps[i][:],
                                w_tile[:, i * P:(i + 1) * P], ct_sb[:, kc],
                                start=(kc == 0), stop=(kc == KD - 1))
                    for i in range(4):
                        m = mg * 4 + i
                        nc.vector.tensor_add(y1_sb[:, m], a_ps[i][:], xts_sb[:, m])

                # ---- LN1 ----
                layernorm(nc, tc, (smallp, sqp, bcp), y1_sb, g1_sb, be1_sb,
                          z1_sb, ones, "ln1")

            # ---- FFN1 + FFN2: the first output half of FFN2 (mg0) is
            # interleaved into the FFN1 loop so W2 @ h starts consuming h
            # chunks as soon as they exist; tile sizes stay [128,512] so the
            # DMA instruction count is unchanged. 8 PSUM banks: 4 h + 4 f. ----
            with tc.tile_pool(name="psa2", bufs=1, space="PSUM") as psa2:
                f_ps = [psa2.tile([P, 512], F32, name=f"f_ps{i}")
                        for i in range(4)]
                for fg in range(KF // 4):
                    h_ps = [psa2.tile([P, 512], F32, name=f"h_ps{i}")
                            for i in range(4)]
                    for kc in range(KD):
                        w_tile = wp.tile([P, 512], F32R, name="w1_tile")
                        nc.sync.dma_start(
                            w_tile[:],
                            w1t[kc * P:(kc + 1) * P, fg * 512:(fg + 1) * 512])
                        for i in range(4):
                            _mm(nc, h_ps[i][:],
                                w_tile[:, i * P:(i + 1) * P], z1_sb[:, kc],
                                start=(kc == 0), stop=(kc == KD - 1))
                    for i in range(4):
                        fm = fg * 4 + i
                        nc.scalar.activation(h_sb[:, fm], h_ps[i][:], AF.Relu,
                                             bias=b1t_sb[:, fm:fm + 1])
                    for i in range(4):
                        fk = fg * 4 + i
                        w_tile = wp.tile([P, 512], F32R, name="w2_tile")
                        nc.sync.dma_start(
                            w_tile[:], w2t[fk * P:(fk + 1) * P, 0:512])
                        for j in range(4):
                            _mm(nc, f_ps[j][:],
                                w_tile[:, j * P:(j + 1) * P], h_sb[:, fk],
                                start=(fk == 0), stop=(fk == KF - 1))
                for j in range(4):
                    nc.vector.scalar_tensor_tensor(
                        out=y2_sb[:, j], in0=f_ps[j][:],
                        scalar=b2t_sb[:, j:j + 1], in1=z1_sb[:, j],
                        op0=OP.add, op1=OP.add)

                f_ps2 = [psa2.tile([P, 512], F32, name=f"f_ps{i}")
                         for i in range(4)]
                for fk in range(KF):
                    w_tile = wp.tile([P, 512], F32R, name="w2_tile")
                    nc.sync.dma_start(
                        w_tile[:], w2t[fk * P:(fk + 1) * P, 512:1024])
                    for j in range(4):
                        _mm(nc, f_ps2[j][:],
                            w_tile[:, j * P:(j + 1) * P], h_sb[:, fk],
                            start=(fk == 0), stop=(fk == KF - 1))
                for j in range(4):
                    m = 4 + j
                    nc.vector.scalar_tensor_tensor(
                        out=y2_sb[:, m], in0=f_ps2[j][:],
                        scalar=b2t_sb[:, m:m + 1], in1=z1_sb[:, m],
                        op0=OP.add, op1=OP.add)

            # ---- LN2 ----
            layernorm(nc, tc, (smallp, sqp, bcp), y2_sb, g2_sb, be2_sb,
                      z2_sb, ones, "ln2")

            for kc in range(KD):
                nc.sync.dma_start(out_t[kc * P:(kc + 1) * P, :], z2_sb[:, kc])
    nc.compile()
    return nc


def _get(name, builder):
    if name not in _CACHE:
        _CACHE[name] = builder()
    return _CACHE[name]


def _prep_inputs(X, Wq, Wk, Wo, ln1_g, ln1_b, ln2_g, ln2_b, W1, b1, W2, b2):
    """Host-side sharding/layout. Returns (in_maps_a, in_maps_b_builder, Xt)."""
    f = lambda a: np.ascontiguousarray(np.asarray(a, dtype=np.float32))
    Xt = f(np.asarray(X, np.float32).reshape(N, D).T)        # [D, N]
    WqT, WkT, WoT = f(np.asarray(Wq).T), f(np.asarray(Wk).T), f(np.asarray(Wo).T)
    W1T, W2T = f(np.asarray(W1).T), f(np.asarray(W2).T)      # [D,FF], [FF,D]
    vecP = lambda v, k: f(np.asarray(v).reshape(k, P).T)     # [P, k]
    g1v, be1v = vecP(ln1_g, D // P), vecP(ln1_b, D // P)
    g2v, be2v = vecP(ln2_g, D // P), vecP(ln2_b, D // P)
    b1v, b2v = vecP(b1, FF // P), vecP(b2, D // P)

    idm = np.tile(np.eye(DH, dtype=np.float32), (2, 1))   # [128, 64]
    in_maps_a = [
        {
            "xt": Xt,
            "idm": idm,
            "wqt": f(WqT[:, c * P:(c + 1) * P]),
            "wkt": f(WkT[:, c * P:(c + 1) * P]),
            "wvt": f(WoT[:, c * P:(c + 1) * P]),
        }
        for c in range(N_CORES)
    ]

    def in_maps_b(ct_full):
        return [
            {
                "ct": f(ct_full[:, c * QC:(c + 1) * QC]),
                "xts": f(Xt[:, c * QC:(c + 1) * QC]),
                "wot": WoT, "w1t": W1T, "w2t": W2T,
                "g1": g1v, "be1": be1v, "g2": g2v, "be2": be2v,
                "b1t": b1v, "b2t": b2v,
            }
            for c in range(N_CORES)
        ]

    return in_maps_a, in_maps_b


def kernel(X, Wq, Wk, Wo, ln1_g, ln1_b, ln2_g, ln2_b, W1, b1, W2, b2):
    in_maps_a, in_maps_b = _prep_inputs(
        X, Wq, Wk, Wo, ln1_g, ln1_b, ln2_g, ln2_b, W1, b1, W2, b2)

    nc_a = _get("a", _build_phase_a)
    res_a = run_bass_kernel_spmd(nc_a, in_maps_a, core_ids=list(range(N_CORES)))
    ct_full = np.concatenate(
        [res_a.results[c]["ctx_t"] for c in range(N_CORES)], axis=0)  # [D, N]

    nc_b = _get("b", _build_phase_b)
    res_b = run_bass_kernel_spmd(nc_b, in_maps_b(ct_full),
                                 core_ids=list(range(N_CORES)))
    out_t = np.concatenate(
        [res_b.results[c]["out_t"] for c in range(N_CORES)], axis=1)  # [D, N]
    return np.ascontiguousarray(out_t.T).reshape(B, S, D).astype(np.float32)



# revision 42
# speedup vs baseline: 1.2533x; 1.2533x over previous
"""Trainium2 Bass kernel for nn_Encoder (dense transformer block), 8 NeuronCores.

Strategy (single chip, 8 cores), v2 with fp8 DoubleRow matmuls:
  Phase A (head-parallel): core c computes attention for heads {2c, 2c+1}.
    Q/K projections run as fp8e4 DoubleRow matmuls (256-deep contraction per
    instruction at 0.5 cyc/row) off an SBUF-resident fp8 X^T; V is projected
    directly into natural [token, dim] layout (also fp8-DR), removing the PE
    transposes. Scores stay fp32r; softmax(relu(s)) = max(exp(s), 1) with a
    bf16 slab, and ctx accumulates in natural layout ([128-token, 65] tiles,
    bf16 at 65 cyc/instr) so the row-sum normalize is a per-partition scalar
    multiply. Output is natural-layout f32 ctx.
  Phase B (row-parallel): core c takes 512 of the 4096 token rows.
    Wo-projection runs fp8-DR off a x16-scaled fp8 ctx; the FFN runs
    error-compensated fp8-DR (x = hi + lo split of both activations and
    weights, keeping the hi@hi + lo@hi + hi@lo terms: 0.75 cyc/row at
    ~bf16 accuracy). LayerNorm reductions run over the partition dim via
    ones-vector matmuls on the PE as before.

kernel() is self-contained: it compiles both phase programs on first call
(cached in module globals) and runs them via run_bass_kernel_spmd.
"""

import os
import sys

for _p in ("/opt/trn_rl_repo",):
    if _p not in sys.path:
        sys.path.insert(0, _p)

# The Bass SPMD path executes through jax/PJRT on the axon platform; make
# sure a caller-pinned JAX_PLATFORMS=cpu doesn't hide the NeuronCores.
_jp = os.environ.get("JAX_PLATFORMS")
if _jp is not None and "axon" not in _jp:
    os.environ["JAX_PLATFORMS"] = "axon," + _jp

import ml_dtypes
import numpy as np

import concourse.bass as bass
import concourse.mybir as mybir
import concourse.tile as tile
from concourse import bacc
from concourse.bass_utils import run_bass_kernel_spmd

F32 = mybir.dt.float32
F32R = mybir.dt.float32r
BF16 = mybir.dt.bfloat16
F8 = mybir.dt.float8e4
AF = mybir.ActivationFunctionType
OP = mybir.AluOpType
DR = mybir.MatmulPerfMode.DoubleRow

NP_F8 = ml_dtypes.float8_e4m3

N_CORES = 8
B, S, D, H, DH, FF = 2, 2048, 1024, 16, 64, 4096
N = B * S            # 4096 token rows
P = 128
QC = N // N_CORES    # 512 rows per core in phase B
KD2 = D // 256       # 4 DoubleRow contraction pairs over D
KF2 = FF // 256      # 16 DoubleRow contraction pairs over FF
KI = S // P          # 16 key chunks of 128 per batch
EPS = 1e-5

_CACHE = {}


def _mm(nc, out, lhsT, rhs, **kw):
    # fp32r: 1-pass FP22 matmul
    nc.tensor.matmul(out, lhsT.bitcast(F32R), rhs.bitcast(F32R), **kw)


# --------------------------------------------------------------------------
# Phase A: per-core head-parallel attention.
# Inputs (per core):
#   xt8  [P, KD2, 2, N]  fp8 X^T in DoubleRow layout: [p,j,i,n] = X[n, d],
#                        d = (2j+i)*128 + p
#   wq8/wk8/wv8 [P, KD2, 2, P]  fp8 16*W{q,k,o}^T columns for this core's two
#                        heads, same d-mapping on the partition side
# Output:
#   ctx_n [N, P] f32  natural-layout ctx: row n, cols 0:64 head0, 64:128 head1
# --------------------------------------------------------------------------
def _build_phase_a():
    nc = bacc.Bacc("TRN2", target_bir_lowering=False, debug=False,
                   num_devices=N_CORES)
    xt8 = nc.dram_tensor("xt8", [P, KD2, 2, N], F8, kind="ExternalInput")
    wq8 = nc.dram_tensor("wq8", [P, KD2, 2, P], F8, kind="ExternalInput")
    wk8 = nc.dram_tensor("wk8", [P, KD2, 2, P], F8, kind="ExternalInput")
    wv8 = nc.dram_tensor("wv8", [P, KD2, 2, P], F8, kind="ExternalInput")
    ctx_n = nc.dram_tensor("ctx_n", [N, P], F32, kind="ExternalOutput")

    NO = S // 512      # 4 query slices of 512 per batch

    with tile.TileContext(nc) as tc:
        with tc.tile_pool(name="persist", bufs=1) as persist:
            xt_sb = persist.tile([P, KD2, 2, N], F8)
            wq_sb = persist.tile([P, KD2, 2, P], F8)
            wk_sb = persist.tile([P, KD2, 2, P], F8)
            wv_sb = persist.tile([P, KD2, 2, P], F8)
            qt_sb = [persist.tile([P, S], F32R, name=f"qt{b_}") for b_ in range(B)]
            kt_sb = [persist.tile([P, S], F32R, name=f"kt{b_}") for b_ in range(B)]
            # V' natural layout per ki chunk: [v_h0(64) | 1 | v_h1(64) | 1]
            vp_sb = [persist.tile([P, KI, 130], BF16, name=f"vp{b_}")
                     for b_ in range(B)]

            # HWDGE descriptor generation serializes DMAs (~625ns each), so
            # order by first use: X^T chunk 0, the q/k weights, then the rest.
            nc.sync.dma_start(xt_sb[:, :, :, 0:512], xt8[:, :, :, 0:512])
            nc.sync.dma_start(wq_sb[:], wq8.ap())
            nc.sync.dma_start(wk_sb[:], wk8.ap())
            nc.sync.dma_start(wv_sb[:], wv8.ap())
            for o8 in range(1, 8):
                nc.sync.dma_start(xt_sb[:, :, :, o8 * 512:(o8 + 1) * 512],
                                  xt8[:, :, :, o8 * 512:(o8 + 1) * 512])
            for b_ in range(B):
                nc.vector.memset(vp_sb[b_][:, :, 64:65], 1.0)
                nc.vector.memset(vp_sb[b_][:, :, 129:130], 1.0)

            with (
                # shared PSUM ring: scores tiles, projection accumulators
                tc.tile_pool(name="ring", bufs=3, space="PSUM") as ring,
                tc.tile_pool(name="cpool", bufs=1, space="PSUM") as cpool,
                tc.tile_pool(name="slabp", bufs=26) as slabp,
                tc.tile_pool(name="coutp", bufs=3) as coutp,
                tc.tile_pool(name="rcp", bufs=4) as rcp,
            ):
                def proj_qk(b_, o):
                    """Project one 512-token slice of batch b_ into qt/kt
                    (t-layout, fp8 DoubleRow, values are 16*q / 16*k)."""
                    sl = slice(b_ * S + o * 512, b_ * S + (o + 1) * 512)
                    for w_sb, dst in ((wq_sb, qt_sb[b_]), (wk_sb, kt_sb[b_])):
                        acc = ring.tile([P, 2, 512], F32, name="rps")
                        for j in range(KD2):
                            nc.tensor.matmul(
                                acc[:, 0, :], w_sb[:, j], xt_sb[:, j, :, sl],
                                start=(j == 0), stop=(j == KD2 - 1),
                                perf_mode=DR)
                        nc.vector.tensor_copy(
                            dst[:, o * 512:(o + 1) * 512], acc[:, 0, :])

                def proj_v(b_, ki):
                    """Project one 128-token chunk of V directly into natural
                    layout (rows = tokens): vp[:, ki, 0:64]=16*v_h0,
                    [65:129]=16*v_h1."""
                    t0 = b_ * S + ki * P
                    acc = ring.tile([P, 2, 512], F32, name="rps")
                    for j in range(KD2):
                        nc.tensor.matmul(
                            acc[:, 0, 0:P], xt_sb[:, j, :, t0:t0 + P],
                            wv_sb[:, j], start=(j == 0), stop=(j == KD2 - 1),
                            perf_mode=DR)
                    for hh in range(2):
                        nc.vector.tensor_copy(
                            vp_sb[b_][:, ki, hh * 65:hh * 65 + 64],
                            acc[:, 0, hh * 64:(hh + 1) * 64])

                chunks = [(b_, o) for b_ in range(B) for o in range(NO)]
                slabs = {}
                cnat = {}

                def unit(idx, pr, h):
                    """Scores + exp + max for (query chunk idx, key pair pr,
                    head h): 2x[128,512] fp32r scores, one exp over 1024."""
                    b_, o = chunks[idx]
                    qs = slice(o * 512, (o + 1) * 512)
                    hs = slice(h * DH, (h + 1) * DH)
                    sp = ring.tile([P, 2, 512], F32, name="rps")
                    for i2 in range(2):
                        ks = (pr * 2 + i2) * P
                        _mm(nc, sp[:, i2, :], kt_sb[b_][hs, ks:ks + P],
                            qt_sb[b_][hs, qs], start=True, stop=True)
                    slab = slabp.tile([P, 2, 512], BF16, name="slab")
                    # scores carry 16*16=256 scaling; true softmax scale 1/8
                    nc.scalar.activation(slab[:], sp[:], AF.Exp,
                                         scale=1.0 / 2048.0)
                    nc.vector.tensor_scalar_max(slab[:], slab[:], 1.0)
                    slabs[(idx, pr, h)] = slab

                def ctx_slot(idx, t, h):
                    """ctx accumulation for one [128-token, 65] output slot.
                    The 32 matmuls of each slot run as one CONTIGUOUS
                    accumulation group: hardware PSUM accumulation breaks
                    when groups at different offsets of the same bank
                    interleave (one accumulation per interruption is lost)."""
                    b_, o = chunks[idx]
                    c = cnat[idx]
                    for pr in range(KI // 2):
                        slab = slabs[(idx, pr, h)]
                        for i2 in range(2):
                            ki = pr * 2 + i2
                            nc.tensor.matmul(
                                c[:, t, h, 0:65],
                                slab[:, i2, t * P:(t + 1) * P],
                                vp_sb[b_][:, ki, h * 65:(h + 1) * 65],
                                start=(pr == 0 and i2 == 0),
                                stop=(pr == KI // 2 - 1 and i2 == 1))
                    if t == 3:
                        for pr in range(KI // 2):
                            slabs.pop((idx, pr, h))

                def normalize(idx):
                    b_, o = chunks[idx]
                    c = cnat.pop(idx)
                    row0 = b_ * S + o * 512
                    # all reciprocals first: the dependent scalar-multiplies
                    # then pipeline without per-pair semaphore round-trips
                    rc = rcp.tile([P, 4, 2], F32, name="rc")
                    for t in range(4):
                        for h in range(2):
                            nc.vector.reciprocal(rc[:, t, h:h + 1],
                                                 c[:, t, h, 64:65])
                    for t in range(4):
                        cout = coutp.tile([P, 2, 64], F32, name="cout")
                        for h in range(2):
                            # ctx carries a x16 scale from 16*v
                            nc.vector.tensor_scalar(
                                out=cout[:, h, :], in0=c[:, t, h, 0:64],
                                scalar1=rc[:, t, h:h + 1], scalar2=1.0 / 16.0,
                                op0=OP.mult, op1=OP.mult)
                        nc.sync.dma_start(
                            ctx_n[row0 + t * P:row0 + (t + 1) * P, :],
                            cout[:])

                # ---- schedule: chunk idx-1's eight ctx slot-groups are
                # spread across chunk idx's 16 score units (h0 groups first,
                # then h1); batch-1 projection chains spread across the unit
                # slots so the scalar engine's exp stream never starves at
                # chunk boundaries.
                slots = [(pr, h) for pr in range(KI // 2) for h in range(2)]
                groups = [(t, h) for h in range(2) for t in range(4)]
                from collections import deque

                for o in range(NO):
                    proj_qk(0, o)
                    proj_v(0, o * 4)
                    proj_v(0, o * 4 + 1)
                    for pr in (2 * o, 2 * o + 1):
                        for h in range(2):
                            unit(0, pr, h)
                    proj_v(0, o * 4 + 2)
                    proj_v(0, o * 4 + 3)
                for idx in range(1, len(chunks)):
                    projs = deque()
                    if idx <= NO:
                        o = idx - 1
                        projs.append(lambda o=o: proj_qk(1, o))
                        for kk in range(4):
                            projs.append(
                                lambda k=o * 4 + kk: proj_v(1, k))
                    cnat[idx - 1] = cpool.tile([P, 4, 2, P], F32, name="cnat")
                    for u in range(len(slots)):
                        if projs and u % 3 == 0:
                            projs.popleft()()
                        unit(idx, *slots[u])
                        if u % 2 == 1:
                            ctx_slot(idx - 1, *groups[u // 2])
                    normalize(idx - 1)
                    while projs:
                        projs.popleft()()
                last = len(chunks) - 1
                cnat[last] = cpool.tile([P, 4, 2, P], F32, name="cnat")
                for t, h in groups:
                    ctx_slot(last, t, h)
                normalize(last)
    nc.compile()
    return nc


# --------------------------------------------------------------------------
# Phase B: per-core row-parallel Wo-proj + AddNorm1 + FFN + AddNorm2.
# Inputs (per core, qi = this core's 512 token rows):
#   ct8   [P, KD2, 2, QC] fp8  16*ctx^T slice, DoubleRow layout
#   xts   [D, QC] f32          X^T slice (residual 1)
#   wot8  [P, KD2, 2, D] fp8   16*Wo^T, DoubleRow layout
#   w1h8/w1l8 [P, KD2, 2, FF] fp8   hi/lo split of 16*W1^T
#   w2h8/w2l8 [P, KF2, 2, D]  fp8   hi/lo split of 32*W2^T
#   g1,be1,g2,be2 [P, D//P], b1t [P, FF//P], b2t [P, D//P]
# Output: out_t [D, QC] f32
# --------------------------------------------------------------------------
def _build_phase_b():
    nc = bacc.Bacc("TRN2", target_bir_lowering=False, debug=False,
                   num_devices=N_CORES)
    ct8 = nc.dram_tensor("ct8", [P, KD2, 2, QC], F8, kind="ExternalInput")
    xts = nc.dram_tensor("xts", [D, QC], F32, kind="ExternalInput")
    wot8 = nc.dram_tensor("wot8", [P, KD2, 2, D], F8, kind="ExternalInput")
    w1h8 = nc.dram_tensor("w1h8", [P, KD2, 2, FF], F8, kind="ExternalInput")
    w1l8 = nc.dram_tensor("w1l8", [P, KD2, 2, FF], F8, kind="ExternalInput")
    w2h8 = nc.dram_tensor("w2h8", [P, KF2, 2, D], F8, kind="ExternalInput")
    w2l8 = nc.dram_tensor("w2l8", [P, KF2, 2, D], F8, kind="ExternalInput")
    # g1 | be1 | g2 | be2 | b2t (8 cols each) | b1t (32 cols)
    pars = nc.dram_tensor("pars", [P, 72], F32, kind="ExternalInput")
    out_t = nc.dram_tensor("out_t", [D, QC], F32, kind="ExternalOutput")

    KD = D // P     # 8
    KF = FF // P    # 32

    with tile.TileContext(nc) as tc:
        with (
            tc.tile_pool(name="persist", bufs=1) as persist,
            tc.tile_pool(name="wp", bufs=2) as wp,
            tc.tile_pool(name="hbp", bufs=2) as hbp,
            tc.tile_pool(name="sqp", bufs=3) as sqp,
            tc.tile_pool(name="smallp", bufs=1) as smallp,
            tc.tile_pool(name="bcp", bufs=2) as bcp,
        ):
            ct_sb = persist.tile([P, KD2, 2, QC], F8)
            pars_sb = persist.tile([P, 72], F32)
            wot_sb = persist.tile([P, KD2, 2, D], F8)
            xts_sb = persist.tile([P, KD, QC], F32)
            y1_sb = persist.tile([P, KD, QC], F32R)
            z1_sb = persist.tile([P, KD, QC], F32R)
            z1h_sb = persist.tile([P, KD2, 2, QC], F8)
            z1l_sb = persist.tile([P, KD2, 2, QC], F8)
            hh_sb = persist.tile([P, KF2, 2, QC], F8)
            hl_sb = persist.tile([P, KF2, 2, QC], F8)
            # y2 reuses y1's slot (y1 dead after LN1); z2 reuses xts's (dead
            # after the y1 adds).
            y2_sb = persist.tile([P, KD, QC], F32R, tag="y1_sb")
            z2_sb = persist.tile([P, KD, QC], F32, tag="xts_sb")
            g1_sb = pars_sb[:, 0:8]
            be1_sb = pars_sb[:, 8:16]
            g2_sb = pars_sb[:, 16:24]
            be2_sb = pars_sb[:, 24:32]
            b2t_sb = pars_sb[:, 32:40]
            b1t_sb = pars_sb[:, 40:72]
            ones = persist.tile([P, 1], F32R)

            nc.sync.dma_start(ct_sb[:], ct8.ap())
            # split so the first Wo matmuls start after ~half the transfer
            nc.sync.dma_start(wot_sb[:, :, :, 0:512], wot8[:, :, :, 0:512])
            nc.sync.dma_start(pars_sb[:], pars.ap())
            nc.sync.dma_start(
                xts_sb[:, 0:4],
                xts[0:512, :].rearrange("(kc p) q -> p kc q", p=P))
            nc.sync.dma_start(wot_sb[:, :, :, 512:1024], wot8[:, :, :, 512:1024])
            nc.sync.dma_start(
                xts_sb[:, 4:8],
                xts[512:1024, :].rearrange("(kc p) q -> p kc q", p=P))
            # 1/D so the stats matmuls produce means directly
            nc.vector.memset(ones[:].bitcast(F32), 1.0 / D)
            eps_sb = persist.tile([1, 1], F32)
            nc.vector.memset(eps_sb[:], EPS)
            ng1_sb = persist.tile([P, KD], F32)
            ng2_sb = persist.tile([P, KD], F32)
            nc.vector.tensor_scalar(out=ng1_sb, in0=g1_sb, scalar1=-1.0,
                                    scalar2=None, op0=OP.mult)
            nc.vector.tensor_scalar(out=ng2_sb, in0=g2_sb, scalar1=-1.0,
                                    scalar2=None, op0=OP.mult)

            def ln_finish(st_ps, z_of_kc, tag):
                """Scalar chain of a layernorm. st_ps [1, 1024] holds the
                per-token mean and mean-square directly (the ones vector is
                1/D); EPS rides the sqrt's bias input. Calls
                z_of_kc(kc, rstd_b, ms_b) to emit per-chunk normalizes."""
                mean = st_ps[:, 0:512]
                ex2 = st_ps[:, 512:1024]
                var = smallp.tile([1, 512], F32, name=f"var_{tag}")
                # mean^2 on Act (the DVE can't read two PSUM operands)
                nc.scalar.activation(var[:], mean, AF.Square)
                nc.vector.tensor_sub(var[:], ex2, var[:])
                std = smallp.tile([1, 512], F32, name=f"std_{tag}")
                nc.scalar.activation(std[:], var[:], AF.Sqrt,
                                     bias=eps_sb[:])
                rstd = smallp.tile([1, 512], F32, name=f"rstd_{tag}")
                nc.vector.reciprocal(rstd[:], std[:])
                ms = smallp.tile([1, 512], F32, name=f"ms_{tag}")
                nc.vector.tensor_mul(ms[:], mean, rstd[:])
                rstd_b = bcp.tile([P, 512], F32, name="rstd_b")
                ms_b = bcp.tile([P, 512], F32, name="ms_b")
                nc.gpsimd.partition_broadcast(rstd_b[:], rstd[:])
                nc.gpsimd.partition_broadcast(ms_b[:], ms[:])
                for kc in range(KD):
                    z_of_kc(kc, rstd_b, ms_b)

            # ---- B1: att^T = (16 Wo^T)(16 ctx) via fp8 DR (+ X residual),
            # with LN1 sum/sumsq matmuls interleaved per finished chunk.
            # Matmuls are emitted per-output-tile so each a_ps buffer
            # completes early and its consumer never gates the next tile. ----
            with tc.tile_pool(name="psa", bufs=1, space="PSUM") as psa, \
                 tc.tile_pool(name="psst1", bufs=1, space="PSUM") as psst1:
                st1 = psst1.tile([1, 1024], F32, name="st1")
                for mg in range(2):
                    a_ps = [psa.tile([P, QC], F32, name=f"a_ps{i}")
                            for i in range(4)]
                    for i in range(4):
                        for j in range(KD2):
                            m0 = mg * 512 + i * P
                            nc.tensor.matmul(
                                a_ps[i][:], wot_sb[:, j, :, m0:m0 + P],
                                ct_sb[:, j], start=(j == 0),
                                stop=(j == KD2 - 1), perf_mode=DR)
                    for i in range(4):
                        m = mg * 4 + i
                        nc.vector.scalar_tensor_tensor(
                            out=y1_sb[:, m], in0=a_ps[i][:],
                            scalar=1.0 / 256.0, in1=xts_sb[:, m],
                            op0=OP.mult, op1=OP.add)
                        _mm(nc, st1[:, 0:512], ones[:], y1_sb[:, m],
                            start=(m == 0), stop=(m == KD - 1))
                        sq = sqp.tile([P, QC], F32R, name="sq")
                        # square on the (otherwise idle) scalar engine
                        nc.scalar.activation(sq[:], y1_sb[:, m].bitcast(F32),
                                             AF.Square)
                        _mm(nc, st1[:, 512:1024], ones[:], sq[:],
                            start=(m == 0), stop=(m == KD - 1))

                # ---- LN1 -> z1 (f32) + hi/lo fp8 split.
                # z = (y*g)*rstd + (be - ms*g): the tensor term precomputes
                # on Pool, the hi cast runs on Act — the DVE does 2 big ops
                # plus the lo residual per chunk. ----
                def z1_emit(kc, rstd_b, ms_b):
                    # nm = be - ms*g on the scalar engine (idle here)
                    nm = bcp.tile([P, QC], F32, name="nmsg")
                    nc.scalar.activation(nm[:], ms_b[:], AF.Identity,
                                         scale=ng1_sb[:, kc:kc + 1],
                                         bias=be1_sb[:, kc:kc + 1])
                    t = sqp.tile([P, QC], F32, name="t_ln")
                    nc.vector.scalar_tensor_tensor(
                        out=t[:], in0=y1_sb[:, kc].bitcast(F32),
                        scalar=g1_sb[:, kc:kc + 1], in1=rstd_b[:],
                        op0=OP.mult, op1=OP.mult)
                    nc.vector.tensor_add(z1_sb[:, kc].bitcast(F32), t[:],
                                         nm[:])
                    jh, ih = kc // 2, kc % 2
                    nc.scalar.activation(z1h_sb[:, jh, ih, :],
                                         z1_sb[:, kc].bitcast(F32), AF.Copy)
                    nc.vector.tensor_sub(z1l_sb[:, jh, ih, :],
                                         z1_sb[:, kc].bitcast(F32),
                                         z1h_sb[:, jh, ih, :])
                    # fold the FFN output bias into z1 (its only remaining
                    # consumer is y2 = f + b2 + z1) on the scalar engine
                    nc.scalar.activation(z1_sb[:, kc].bitcast(F32),
                                         z1_sb[:, kc].bitcast(F32),
                                         AF.Identity,
                                         bias=b2t_sb[:, kc:kc + 1])

                ln_finish(st1, z1_emit, "ln1")

            # ---- FFN1 + FFN2(first half) interleaved. fp8 DR with hi/lo
            # error compensation: keep hi@hi + lo@hi + hi@lo. Matmuls are
            # emitted per-output-tile (12 back-to-back accumulations) so
            # each h_ps tile finishes early and its relu overlaps the next
            # tile's matmuls — h_ps stays single-buffered (8 PSUM banks
            # total: 4 h_ps + 4 f_ps). ----
            def y2_emit(m, fp):
                # f_ps = 32*f; y2 = f + (z1 + b2)  (b2 folded into z1)
                nc.vector.scalar_tensor_tensor(
                    out=y2_sb[:, m], in0=fp[:],
                    scalar=1.0 / 32.0, in1=z1_sb[:, m].bitcast(F32),
                    op0=OP.mult, op1=OP.add)

            with tc.tile_pool(name="psf", bufs=1, space="PSUM") as psf:
                f_ps = [psf.tile([P, QC], F32, name=f"f_ps{i}")
                        for i in range(4)]
                for fg in range(KF // 4):
                    w1h_t = wp.tile([P, KD2, 2, 512], F8, name="w1h_t")
                    w1l_t = wp.tile([P, KD2, 2, 512], F8, name="w1l_t")
                    nc.sync.dma_start(w1h_t[:],
                                      w1h8[:, :, :, fg * 512:(fg + 1) * 512])
                    nc.sync.dma_start(w1l_t[:],
                                      w1l8[:, :, :, fg * 512:(fg + 1) * 512])
                    h_ps = [psf.tile([P, QC], F32, name=f"h_ps{i}")
                            for i in range(4)]
                    hb = hbp.tile([P, 4, QC], BF16, name="hb")
                    for i in range(4):
                        # per-tile-major: each tile finishes early so its
                        # relu overlaps the next tile's matmuls
                        for pi, (wt, zt) in enumerate(((w1h_t, z1h_sb),
                                                       (w1l_t, z1h_sb),
                                                       (w1h_t, z1l_sb))):
                            for j in range(KD2):
                                nc.tensor.matmul(
                                    h_ps[i][:],
                                    wt[:, j, :, i * P:(i + 1) * P],
                                    zt[:, j], start=(j == 0 and pi == 0),
                                    stop=(j == KD2 - 1 and pi == 2),
                                    perf_mode=DR)
                        fm = fg * 4 + i
                        # h_ps = 16*(z1@W1^T); relu(x/16 + b1). The hi fp8
                        # copy is a second relu on the scalar engine; the lo
                        # residual runs on Pool — both off the DVE.
                        jp, ip = fm // 2, fm % 2
                        nc.scalar.activation(hb[:, i, :], h_ps[i][:], AF.Relu,
                                             bias=b1t_sb[:, fm:fm + 1],
                                             scale=1.0 / 16.0)
                        nc.scalar.activation(hh_sb[:, jp, ip, :], h_ps[i][:],
                                             AF.Relu,
                                             bias=b1t_sb[:, fm:fm + 1],
                                             scale=1.0 / 16.0)
                        nc.vector.tensor_sub(hl_sb[:, jp, ip, :],
                                             hb[:, i, :],
                                             hh_sb[:, jp, ip, :])
                    # FFN2 first output half: consume h pairs as they finish;
                    # the weight tiles prefetch one fg ahead.
                    if fg % 2 == 0:
                        g4p = fg // 2
                        w2h_t = wp.tile([P, 4, 2, 512], F8, name="w2h_t")
                        w2l_t = wp.tile([P, 4, 2, 512], F8, name="w2l_t")
                        nc.sync.dma_start(
                            w2h_t[:], w2h8[:, g4p * 4:(g4p + 1) * 4, :, 0:512])
                        nc.sync.dma_start(
                            w2l_t[:], w2l8[:, g4p * 4:(g4p + 1) * 4, :, 0:512])
                    if fg % 2 == 1:
                        g4 = fg // 2       # group of 4 jp pairs
                        for i in range(4):
                            for jj in range(4):
                                jp = g4 * 4 + jj
                                for wt, ht, pi in ((w2h_t, hh_sb, 0),
                                                   (w2l_t, hh_sb, 1),
                                                   (w2h_t, hl_sb, 2)):
                                    nc.tensor.matmul(
                                        f_ps[i][:],
                                        wt[:, jj, :, i * P:(i + 1) * P],
                                        ht[:, jp], start=(jp == 0 and pi == 0),
                                        stop=(jp == KF2 - 1 and pi == 2),
                                        perf_mode=DR)
                for i in range(4):
                    y2_emit(i, f_ps[i])

            # FFN2 second output half (fresh PSUM pool after psf closes)
            with tc.tile_pool(name="psf2", bufs=1, space="PSUM") as psf2, \
                 tc.tile_pool(name="psst2", bufs=1, space="PSUM") as psst2:
                st2 = psst2.tile([1, 1024], F32, name="st2")

                def y2_stats(m):
                    _mm(nc, st2[:, 0:512], ones[:], y2_sb[:, m],
                        start=(m == 0), stop=(m == KD - 1))
                    sq = sqp.tile([P, QC], F32R, name="sq")
                    nc.scalar.activation(sq[:], y2_sb[:, m].bitcast(F32),
                                         AF.Square)
                    _mm(nc, st2[:, 512:1024], ones[:], sq[:],
                        start=(m == 0), stop=(m == KD - 1))

                # first-half y2 stats run under the mg1 matmuls
                for m in range(4):
                    y2_stats(m)
                f_ps2 = [psf2.tile([P, QC], F32, name=f"f2_ps{i}")
                         for i in range(4)]
                for g4 in range(4):
                    w2h_t = wp.tile([P, 4, 2, 512], F8, name="w2h_t")
                    w2l_t = wp.tile([P, 4, 2, 512], F8, name="w2l_t")
                    nc.sync.dma_start(
                        w2h_t[:], w2h8[:, g4 * 4:(g4 + 1) * 4, :, 512:1024])
                    nc.sync.dma_start(
                        w2l_t[:], w2l8[:, g4 * 4:(g4 + 1) * 4, :, 512:1024])
                    for i in range(4):
                        for jj in range(4):
                            jp = g4 * 4 + jj
                            for wt, ht, pi in ((w2h_t, hh_sb, 0),
                                               (w2l_t, hh_sb, 1),
                                               (w2h_t, hl_sb, 2)):
                                nc.tensor.matmul(
                                    f_ps2[i][:],
                                    wt[:, jj, :, i * P:(i + 1) * P],
                                    ht[:, jp], start=(jp == 0 and pi == 0),
                                    stop=(jp == KF2 - 1 and pi == 2),
                                    perf_mode=DR)
                for i in range(4):
                    y2_emit(4 + i, f_ps2[i])
                    y2_stats(4 + i)

                # ---- LN2 normalize -> z2 -> out. 2 DVE ops per chunk;
                # the (be - ms*g) tensor precomputes on Pool. ----
                def z2_emit(kc, rstd_b, ms_b):
                    nm = bcp.tile([P, QC], F32, name="nmsg")
                    nc.scalar.activation(nm[:], ms_b[:], AF.Identity,
                                         scale=ng2_sb[:, kc:kc + 1],
                                         bias=be2_sb[:, kc:kc + 1])
                    t = sqp.tile([P, QC], F32, name=f"t_ln{kc % 2}")
                    nc.vector.scalar_tensor_tensor(
                        out=t[:], in0=y2_sb[:, kc].bitcast(F32),
                        scalar=g2_sb[:, kc:kc + 1], in1=rstd_b[:],
                        op0=OP.mult, op1=OP.mult)
                    nc.vector.tensor_add(z2_sb[:, kc], t[:], nm[:])
                    nc.sync.dma_start(out_t[kc * P:(kc + 1) * P, :],
                                      z2_sb[:, kc])

                ln_finish(st2, z2_emit, "ln2")
    nc.compile()
    return nc


def _get(name, builder):
    if name not in _CACHE:
        _CACHE[name] = builder()
    return _CACHE[name]


def _dr_pack(a, scale=1.0):
    """[Din, M] f32 -> [P, Din//256, 2, M] fp8 DoubleRow layout."""
    a = np.asarray(a, np.float32) * scale
    din, m = a.shape
    return np.ascontiguousarray(
        a.reshape(din // 256, 2, P, m).transpose(2, 0, 1, 3).astype(NP_F8))


def _prep_inputs(X, Wq, Wk, Wo, ln1_g, ln1_b, ln2_g, ln2_b, W1, b1, W2, b2):
    f = lambda a: np.ascontiguousarray(np.asarray(a, dtype=np.float32))
    Xf = np.asarray(X, np.float32).reshape(N, D)
    Xt = f(Xf.T)                                              # [D, N]
    xt8 = _dr_pack(Xt)                                        # [P,4,2,N] fp8
    WqT, WkT, WoT = (np.asarray(w, np.float32).T for w in (Wq, Wk, Wo))
    vecP = lambda v, k: f(np.asarray(v).reshape(k, P).T)      # [P, k]
    parsv = np.concatenate(
        [vecP(ln1_g, D // P), vecP(ln1_b, D // P), vecP(ln2_g, D // P),
         vecP(ln2_b, D // P), vecP(b2, D // P), vecP(b1, FF // P)], axis=1)

    in_maps_a = [
        {
            "xt8": xt8,
            "wq8": _dr_pack(WqT[:, c * P:(c + 1) * P], 16.0),
            "wk8": _dr_pack(WkT[:, c * P:(c + 1) * P], 16.0),
            "wv8": _dr_pack(WoT[:, c * P:(c + 1) * P], 16.0),
        }
        for c in range(N_CORES)
    ]

    wot8 = _dr_pack(WoT, 16.0)
    W1sT = np.asarray(W1, np.float32).T * 16.0                # [D, FF]
    w1h = W1sT.astype(NP_F8)
    w1l = (W1sT - w1h.astype(np.float32)).astype(NP_F8)
    W2sT = np.asarray(W2, np.float32).T * 32.0                # [FF, D]
    w2h = W2sT.astype(NP_F8)
    w2l = (W2sT - w2h.astype(np.float32)).astype(NP_F8)
    pack8 = lambda a: np.ascontiguousarray(
        a.reshape(a.shape[0] // 256, 2, P, a.shape[1]).transpose(2, 0, 1, 3))
    w1h8, w1l8 = pack8(w1h), pack8(w1l)
    w2h8, w2l8 = pack8(w2h), pack8(w2l)

    def in_maps_b(ctx_full):
        # ctx_full: [N, D] f32 natural layout
        ct_t = ctx_full.T                                     # [D, N]
        return [
            {
                "ct8": _dr_pack(ct_t[:, c * QC:(c + 1) * QC], 16.0),
                "xts": f(Xt[:, c * QC:(c + 1) * QC]),
                "wot8": wot8,
                "w1h8": w1h8, "w1l8": w1l8,
                "w2h8": w2h8, "w2l8": w2l8,
                "pars": parsv,
            }
            for c in range(N_CORES)
        ]

    return in_maps_a, in_maps_b


def kernel(X, Wq, Wk, Wo, ln1_g, ln1_b, ln2_g, ln2_b, W1, b1, W2, b2):
    in_maps_a, in_maps_b = _prep_inputs(
        X, Wq, Wk, Wo, ln1_g, ln1_b, ln2_g, ln2_b, W1, b1, W2, b2)

    nc_a = _get("a", _build_phase_a)
    res_a = run_bass_kernel_spmd(nc_a, in_maps_a, core_ids=list(range(N_CORES)))
    # ctx_n[c] is [N, 128]: head-block columns for heads (2c, 2c+1)
    ctx_full = np.concatenate(
        [res_a.results[c]["ctx_n"] for c in range(N_CORES)], axis=1)  # [N, D]

    nc_b = _get("b", _build_phase_b)
    res_b = run_bass_kernel_spmd(nc_b, in_maps_b(ctx_full),
                                 core_ids=list(range(N_CORES)))
    out_t = np.concatenate(
        [res_b.results[c]["out_t"] for c in range(N_CORES)], axis=1)  # [D, N]
    return np.ascontiguousarray(out_t.T).reshape(B, S, D).astype(np.float32)


# revision 44
# speedup vs baseline: 1.2538x; 1.0004x over previous
"""Trainium2 Bass kernel for nn_Encoder (dense transformer block), 8 NeuronCores.

Strategy (single chip, 8 cores), v2 with fp8 DoubleRow matmuls:
  Phase A (head-parallel): core c computes attention for heads {2c, 2c+1}.
    Q/K projections run as fp8e4 DoubleRow matmuls (256-deep contraction per
    instruction at 0.5 cyc/row) off an SBUF-resident fp8 X^T; V is projected
    directly into natural [token, dim] layout (also fp8-DR), removing the PE
    transposes. Scores stay fp32r; softmax(relu(s)) = max(exp(s), 1) with a
    bf16 slab, and ctx accumulates in natural layout ([128-token, 65] tiles,
    bf16 at 65 cyc/instr) so the row-sum normalize is a per-partition scalar
    multiply. Output is natural-layout f32 ctx.
  Phase B (row-parallel): core c takes 512 of the 4096 token rows.
    Wo-projection runs fp8-DR off a x16-scaled fp8 ctx; the FFN runs
    error-compensated fp8-DR (x = hi + lo split of both activations and
    weights, keeping the hi@hi + lo@hi + hi@lo terms: 0.75 cyc/row at
    ~bf16 accuracy). LayerNorm reductions run over the partition dim via
    ones-vector matmuls on the PE as before.

kernel() is self-contained: it compiles both phase programs on first call
(cached in module globals) and runs them via run_bass_kernel_spmd.
"""

import os
import sys

for _p in ("/opt/trn_rl_repo",):
    if _p not in sys.path:
        sys.path.insert(0, _p)

# The Bass SPMD path executes through jax/PJRT on the axon platform; make
# sure a caller-pinned JAX_PLATFORMS=cpu doesn't hide the NeuronCores.
_jp = os.environ.get("JAX_PLATFORMS")
if _jp is not None and "axon" not in _jp:
    os.environ["JAX_PLATFORMS"] = "axon," + _jp

import ml_dtypes
import numpy as np

import concourse.bass as bass
import concourse.mybir as mybir
import concourse.tile as tile
from concourse import bacc
from concourse.bass_utils import run_bass_kernel_spmd

F32 = mybir.dt.float32
F32R = mybir.dt.float32r
BF16 = mybir.dt.bfloat16
F8 = mybir.dt.float8e4
AF = mybir.ActivationFunctionType
OP = mybir.AluOpType
DR = mybir.MatmulPerfMode.DoubleRow

NP_F8 = ml_dtypes.float8_e4m3

N_CORES = 8
B, S, D, H, DH, FF = 2, 2048, 1024, 16, 64, 4096
N = B * S            # 4096 token rows
P = 128
QC = N // N_CORES    # 512 rows per core in phase B
KD2 = D // 256       # 4 DoubleRow contraction pairs over D
KF2 = FF // 256      # 16 DoubleRow contraction pairs over FF
KI = S // P          # 16 key chunks of 128 per batch
EPS = 1e-5

_CACHE = {}


def _mm(nc, out, lhsT, rhs, **kw):
    # fp32r: 1-pass FP22 matmul
    nc.tensor.matmul(out, lhsT.bitcast(F32R), rhs.bitcast(F32R), **kw)


# --------------------------------------------------------------------------
# Phase A: per-core head-parallel attention.
# Inputs (per core):
#   xt8  [P, KD2, 2, N]  fp8 X^T in DoubleRow layout: [p,j,i,n] = X[n, d],
#                        d = (2j+i)*128 + p
#   wq8/wk8/wv8 [P, KD2, 2, P]  fp8 16*W{q,k,o}^T columns for this core's two
#                        heads, same d-mapping on the partition side
# Output:
#   ctx_n [N, P] f32  natural-layout ctx: row n, cols 0:64 head0, 64:128 head1
# --------------------------------------------------------------------------
def _build_phase_a():
    nc = bacc.Bacc("TRN2", target_bir_lowering=False, debug=False,
                   num_devices=N_CORES)
    xt8 = nc.dram_tensor("xt8", [P, KD2, 2, N], F8, kind="ExternalInput")
    wq8 = nc.dram_tensor("wq8", [P, KD2, 2, P], F8, kind="ExternalInput")
    wk8 = nc.dram_tensor("wk8", [P, KD2, 2, P], F8, kind="ExternalInput")
    wv8 = nc.dram_tensor("wv8", [P, KD2, 2, P], F8, kind="ExternalInput")
    ctx_n = nc.dram_tensor("ctx_n", [N, P], F32, kind="ExternalOutput")

    NO = S // 512      # 4 query slices of 512 per batch

    with tile.TileContext(nc) as tc:
        with tc.tile_pool(name="persist", bufs=1) as persist:
            xt_sb = persist.tile([P, KD2, 2, N], F8)
            wq_sb = persist.tile([P, KD2, 2, P], F8)
            wk_sb = persist.tile([P, KD2, 2, P], F8)
            wv_sb = persist.tile([P, KD2, 2, P], F8)
            qt_sb = [persist.tile([P, S], F32R, name=f"qt{b_}") for b_ in range(B)]
            kt_sb = [persist.tile([P, S], F32R, name=f"kt{b_}") for b_ in range(B)]
            # V' natural layout per ki chunk: [v_h0(64) | 1 | v_h1(64) | 1]
            vp_sb = [persist.tile([P, KI, 130], BF16, name=f"vp{b_}")
                     for b_ in range(B)]

            # HWDGE descriptor generation serializes DMAs (~625ns each), so
            # order by first use: X^T chunk 0, the q/k weights, then the rest.
            nc.sync.dma_start(xt_sb[:, :, :, 0:512], xt8[:, :, :, 0:512])
            nc.sync.dma_start(wq_sb[:], wq8.ap())
            nc.sync.dma_start(wk_sb[:], wk8.ap())
            nc.sync.dma_start(wv_sb[:], wv8.ap())
            for o8 in range(1, 8):
                nc.sync.dma_start(xt_sb[:, :, :, o8 * 512:(o8 + 1) * 512],
                                  xt8[:, :, :, o8 * 512:(o8 + 1) * 512])
            for b_ in range(B):
                nc.vector.memset(vp_sb[b_][:, :, 64:65], 1.0)
                nc.vector.memset(vp_sb[b_][:, :, 129:130], 1.0)

            with (
                # shared PSUM ring: scores tiles, projection accumulators
                tc.tile_pool(name="ring", bufs=3, space="PSUM") as ring,
                tc.tile_pool(name="cpool", bufs=1, space="PSUM") as cpool,
                tc.tile_pool(name="slabp", bufs=26) as slabp,
                tc.tile_pool(name="coutp", bufs=3) as coutp,
                tc.tile_pool(name="rcp", bufs=4) as rcp,
            ):
                def proj_qk(b_, o):
                    """Project one 512-token slice of batch b_ into qt/kt
                    (t-layout, fp8 DoubleRow, values are 16*q / 16*k)."""
                    sl = slice(b_ * S + o * 512, b_ * S + (o + 1) * 512)
                    for w_sb, dst in ((wq_sb, qt_sb[b_]), (wk_sb, kt_sb[b_])):
                        acc = ring.tile([P, 2, 512], F32, name="rps")
                        for j in range(KD2):
                            nc.tensor.matmul(
                                acc[:, 0, :], w_sb[:, j], xt_sb[:, j, :, sl],
                                start=(j == 0), stop=(j == KD2 - 1),
                                perf_mode=DR)
                        nc.vector.tensor_copy(
                            dst[:, o * 512:(o + 1) * 512], acc[:, 0, :])

                def proj_v(b_, ki):
                    """Project one 128-token chunk of V directly into natural
                    layout (rows = tokens): vp[:, ki, 0:64]=16*v_h0,
                    [65:129]=16*v_h1."""
                    t0 = b_ * S + ki * P
                    acc = ring.tile([P, 2, 512], F32, name="rps")
                    for j in range(KD2):
                        nc.tensor.matmul(
                            acc[:, 0, 0:P], xt_sb[:, j, :, t0:t0 + P],
                            wv_sb[:, j], start=(j == 0), stop=(j == KD2 - 1),
                            perf_mode=DR)
                    for hh in range(2):
                        nc.vector.tensor_copy(
                            vp_sb[b_][:, ki, hh * 65:hh * 65 + 64],
                            acc[:, 0, hh * 64:(hh + 1) * 64])

                chunks = [(b_, o) for b_ in range(B) for o in range(NO)]
                slabs = {}
                cnat = {}

                def unit(idx, pr, h, split=False):
                    """Scores + exp + max for (query chunk idx, key pair pr,
                    head h): 2x[128,512] fp32r scores, one exp over 1024.
                    split=True emits per-query-half score/exp pairs (same
                    values) so the first exp starts earlier at startup."""
                    b_, o = chunks[idx]
                    hs = slice(h * DH, (h + 1) * DH)
                    sp = ring.tile([P, 2, 512], F32, name="rps")
                    slab = slabp.tile([P, 2, 512], BF16, name="slab")
                    halves = ((0, 256), (256, 512)) if split else ((0, 512),)
                    for q0, q1 in halves:
                        qs = slice(o * 512 + q0, o * 512 + q1)
                        for i2 in range(2):
                            ks = (pr * 2 + i2) * P
                            _mm(nc, sp[:, i2, q0:q1], kt_sb[b_][hs, ks:ks + P],
                                qt_sb[b_][hs, qs], start=True, stop=True)
                        # scores carry 16*16=256 scaling; softmax scale 1/8
                        nc.scalar.activation(slab[:, :, q0:q1],
                                             sp[:, :, q0:q1], AF.Exp,
                                             scale=1.0 / 2048.0)
                        nc.vector.tensor_scalar_max(slab[:, :, q0:q1],
                                                    slab[:, :, q0:q1], 1.0)
                    slabs[(idx, pr, h)] = slab

                def ctx_slot(idx, t, h):
                    """ctx accumulation for one [128-token, 65] output slot.
                    The 32 matmuls of each slot run as one CONTIGUOUS
                    accumulation group: hardware PSUM accumulation breaks
                    when groups at different offsets of the same bank
                    interleave (one accumulation per interruption is lost)."""
                    b_, o = chunks[idx]
                    c = cnat[idx]
                    for pr in range(KI // 2):
                        slab = slabs[(idx, pr, h)]
                        for i2 in range(2):
                            ki = pr * 2 + i2
                            nc.tensor.matmul(
                                c[:, t, h, 0:65],
                                slab[:, i2, t * P:(t + 1) * P],
                                vp_sb[b_][:, ki, h * 65:(h + 1) * 65],
                                start=(pr == 0 and i2 == 0),
                                stop=(pr == KI // 2 - 1 and i2 == 1))
                    if t == 3:
                        for pr in range(KI // 2):
                            slabs.pop((idx, pr, h))

                def normalize(idx):
                    b_, o = chunks[idx]
                    c = cnat.pop(idx)
                    row0 = b_ * S + o * 512
                    # all reciprocals first: the dependent scalar-multiplies
                    # then pipeline without per-pair semaphore round-trips
                    rc = rcp.tile([P, 4, 2], F32, name="rc")
                    for t in range(4):
                        for h in range(2):
                            nc.vector.reciprocal(rc[:, t, h:h + 1],
                                                 c[:, t, h, 64:65])
                    for t in range(4):
                        cout = coutp.tile([P, 2, 64], F32, name="cout")
                        for h in range(2):
                            # ctx carries a x16 scale from 16*v
                            nc.vector.tensor_scalar(
                                out=cout[:, h, :], in0=c[:, t, h, 0:64],
                                scalar1=rc[:, t, h:h + 1], scalar2=1.0 / 16.0,
                                op0=OP.mult, op1=OP.mult)
                        nc.sync.dma_start(
                            ctx_n[row0 + t * P:row0 + (t + 1) * P, :],
                            cout[:])

                # ---- schedule: chunk idx-1's eight ctx slot-groups are
                # spread across chunk idx's 16 score units (h0 groups first,
                # then h1); batch-1 projection chains spread across the unit
                # slots so the scalar engine's exp stream never starves at
                # chunk boundaries.
                slots = [(pr, h) for pr in range(KI // 2) for h in range(2)]
                groups = [(t, h) for h in range(2) for t in range(4)]
                from collections import deque

                for o in range(NO):
                    proj_qk(0, o)
                    proj_v(0, o * 4)
                    proj_v(0, o * 4 + 1)
                    for pr in (2 * o, 2 * o + 1):
                        for h in range(2):
                            unit(0, pr, h)
                    proj_v(0, o * 4 + 2)
                    proj_v(0, o * 4 + 3)
                for idx in range(1, len(chunks)):
                    projs = deque()
                    if idx <= NO:
                        o = idx - 1
                        projs.append(lambda o=o: proj_qk(1, o))
                        for kk in range(4):
                            projs.append(
                                lambda k=o * 4 + kk: proj_v(1, k))
                    cnat[idx - 1] = cpool.tile([P, 4, 2, P], F32, name="cnat")
                    for u in range(len(slots)):
                        if projs and u % 3 == 0:
                            projs.popleft()()
                        unit(idx, *slots[u])
                        if u % 2 == 1:
                            ctx_slot(idx - 1, *groups[u // 2])
                    normalize(idx - 1)
                    while projs:
                        projs.popleft()()
                last = len(chunks) - 1
                cnat[last] = cpool.tile([P, 4, 2, P], F32, name="cnat")
                for t, h in groups:
                    ctx_slot(last, t, h)
                normalize(last)
    nc.compile()
    return nc


# --------------------------------------------------------------------------
# Phase B: per-core row-parallel Wo-proj + AddNorm1 + FFN + AddNorm2.
# Inputs (per core, qi = this core's 512 token rows):
#   ct8   [P, KD2, 2, QC] fp8  16*ctx^T slice, DoubleRow layout
#   xts   [D, QC] f32          X^T slice (residual 1)
#   wot8  [P, KD2, 2, D] fp8   16*Wo^T, DoubleRow layout
#   w1h8/w1l8 [P, KD2, 2, FF] fp8   hi/lo split of 16*W1^T
#   w2h8/w2l8 [P, KF2, 2, D]  fp8   hi/lo split of 32*W2^T
#   g1,be1,g2,be2 [P, D//P], b1t [P, FF//P], b2t [P, D//P]
# Output: out_t [D, QC] f32
# --------------------------------------------------------------------------
def _build_phase_b():
    nc = bacc.Bacc("TRN2", target_bir_lowering=False, debug=False,
                   num_devices=N_CORES)
    ct8 = nc.dram_tensor("ct8", [P, KD2, 2, QC], F8, kind="ExternalInput")
    xts = nc.dram_tensor("xts", [D, QC], F32, kind="ExternalInput")
    wot8 = nc.dram_tensor("wot8", [P, KD2, 2, D], F8, kind="ExternalInput")
    w1h8 = nc.dram_tensor("w1h8", [P, KD2, 2, FF], F8, kind="ExternalInput")
    w1l8 = nc.dram_tensor("w1l8", [P, KD2, 2, FF], F8, kind="ExternalInput")
    w2h8 = nc.dram_tensor("w2h8", [P, KF2, 2, D], F8, kind="ExternalInput")
    w2l8 = nc.dram_tensor("w2l8", [P, KF2, 2, D], F8, kind="ExternalInput")
    # g1 | be1 | g2 | be2 | b2t (8 cols each) | b1t (32 cols)
    pars = nc.dram_tensor("pars", [P, 72], F32, kind="ExternalInput")
    out_t = nc.dram_tensor("out_t", [D, QC], F32, kind="ExternalOutput")

    KD = D // P     # 8
    KF = FF // P    # 32

    with tile.TileContext(nc) as tc:
        with (
            tc.tile_pool(name="persist", bufs=1) as persist,
            tc.tile_pool(name="wp", bufs=2) as wp,
            tc.tile_pool(name="hbp", bufs=2) as hbp,
            tc.tile_pool(name="sqp", bufs=3) as sqp,
            tc.tile_pool(name="smallp", bufs=1) as smallp,
            tc.tile_pool(name="bcp", bufs=2) as bcp,
        ):
            ct_sb = persist.tile([P, KD2, 2, QC], F8)
            pars_sb = persist.tile([P, 72], F32)
            wot_sb = persist.tile([P, KD2, 2, D], F8)
            xts_sb = persist.tile([P, KD, QC], F32)
            y1_sb = persist.tile([P, KD, QC], F32R)
            z1_sb = persist.tile([P, KD, QC], F32R)
            z1h_sb = persist.tile([P, KD2, 2, QC], F8)
            z1l_sb = persist.tile([P, KD2, 2, QC], F8)
            hh_sb = persist.tile([P, KF2, 2, QC], F8)
            hl_sb = persist.tile([P, KF2, 2, QC], F8)
            # y2 reuses y1's slot (y1 dead after LN1); z2 reuses xts's (dead
            # after the y1 adds).
            y2_sb = persist.tile([P, KD, QC], F32R, tag="y1_sb")
            z2_sb = persist.tile([P, KD, QC], F32, tag="xts_sb")
            g1_sb = pars_sb[:, 0:8]
            be1_sb = pars_sb[:, 8:16]
            g2_sb = pars_sb[:, 16:24]
            be2_sb = pars_sb[:, 24:32]
            b2t_sb = pars_sb[:, 32:40]
            b1t_sb = pars_sb[:, 40:72]
            ones = persist.tile([P, 1], F32R)

            nc.sync.dma_start(ct_sb[:], ct8.ap())
            # split so the first Wo matmuls start after ~a quarter of the
            # weight transfer
            nc.sync.dma_start(wot_sb[:, :, :, 0:256], wot8[:, :, :, 0:256])
            nc.sync.dma_start(wot_sb[:, :, :, 256:512], wot8[:, :, :, 256:512])
            nc.sync.dma_start(pars_sb[:], pars.ap())
            nc.sync.dma_start(
                xts_sb[:, 0:4],
                xts[0:512, :].rearrange("(kc p) q -> p kc q", p=P))
            nc.sync.dma_start(wot_sb[:, :, :, 512:1024], wot8[:, :, :, 512:1024])
            nc.sync.dma_start(
                xts_sb[:, 4:8],
                xts[512:1024, :].rearrange("(kc p) q -> p kc q", p=P))
            # 1/D so the stats matmuls produce means directly
            nc.vector.memset(ones[:].bitcast(F32), 1.0 / D)
            eps_sb = persist.tile([1, 1], F32)
            nc.vector.memset(eps_sb[:], EPS)
            ng1_sb = persist.tile([P, KD], F32)
            ng2_sb = persist.tile([P, KD], F32)
            nc.vector.tensor_scalar(out=ng1_sb, in0=g1_sb, scalar1=-1.0,
                                    scalar2=None, op0=OP.mult)
            nc.vector.tensor_scalar(out=ng2_sb, in0=g2_sb, scalar1=-1.0,
                                    scalar2=None, op0=OP.mult)

            def ln_finish(st_ps, z_of_kc, tag):
                """Scalar chain of a layernorm. st_ps [1, 1024] holds the
                per-token mean and mean-square directly (the ones vector is
                1/D); EPS rides the sqrt's bias input. Calls
                z_of_kc(kc, rstd_b, ms_b) to emit per-chunk normalizes."""
                mean = st_ps[:, 0:512]
                ex2 = st_ps[:, 512:1024]
                var = smallp.tile([1, 512], F32, name=f"var_{tag}")
                # mean^2 on Act (the DVE can't read two PSUM operands)
                nc.scalar.activation(var[:], mean, AF.Square)
                nc.vector.tensor_sub(var[:], ex2, var[:])
                std = smallp.tile([1, 512], F32, name=f"std_{tag}")
                nc.scalar.activation(std[:], var[:], AF.Sqrt,
                                     bias=eps_sb[:])
                rstd = smallp.tile([1, 512], F32, name=f"rstd_{tag}")
                nc.vector.reciprocal(rstd[:], std[:])
                ms = smallp.tile([1, 512], F32, name=f"ms_{tag}")
                nc.vector.tensor_mul(ms[:], mean, rstd[:])
                rstd_b = bcp.tile([P, 512], F32, name="rstd_b")
                ms_b = bcp.tile([P, 512], F32, name="ms_b")
                nc.gpsimd.partition_broadcast(rstd_b[:], rstd[:])
                nc.gpsimd.partition_broadcast(ms_b[:], ms[:])
                for kc in range(KD):
                    z_of_kc(kc, rstd_b, ms_b)

            # ---- B1: att^T = (16 Wo^T)(16 ctx) via fp8 DR (+ X residual),
            # with LN1 sum/sumsq matmuls interleaved per finished chunk.
            # Matmuls are emitted per-output-tile so each a_ps buffer
            # completes early and its consumer never gates the next tile. ----
            with tc.tile_pool(name="psa", bufs=1, space="PSUM") as psa, \
                 tc.tile_pool(name="psst1", bufs=1, space="PSUM") as psst1:
                st1 = psst1.tile([1, 1024], F32, name="st1")
                for mg in range(2):
                    a_ps = [psa.tile([P, QC], F32, name=f"a_ps{i}")
                            for i in range(4)]
                    for i in range(4):
                        for j in range(KD2):
                            m0 = mg * 512 + i * P
                            nc.tensor.matmul(
                                a_ps[i][:], wot_sb[:, j, :, m0:m0 + P],
                                ct_sb[:, j], start=(j == 0),
                                stop=(j == KD2 - 1), perf_mode=DR)
                    for i in range(4):
                        m = mg * 4 + i
                        nc.vector.scalar_tensor_tensor(
                            out=y1_sb[:, m], in0=a_ps[i][:],
                            scalar=1.0 / 256.0, in1=xts_sb[:, m],
                            op0=OP.mult, op1=OP.add)
                        _mm(nc, st1[:, 0:512], ones[:], y1_sb[:, m],
                            start=(m == 0), stop=(m == KD - 1))
                        sq = sqp.tile([P, QC], F32R, name="sq")
                        # square on the (otherwise idle) scalar engine
                        nc.scalar.activation(sq[:], y1_sb[:, m].bitcast(F32),
                                             AF.Square)
                        _mm(nc, st1[:, 512:1024], ones[:], sq[:],
                            start=(m == 0), stop=(m == KD - 1))

                # ---- LN1 -> z1 (f32) + hi/lo fp8 split.
                # z = (y*g)*rstd + (be - ms*g): the tensor term precomputes
                # on Pool, the hi cast runs on Act — the DVE does 2 big ops
                # plus the lo residual per chunk. ----
                def z1_emit(kc, rstd_b, ms_b):
                    # nm = be - ms*g on the scalar engine (idle here)
                    nm = bcp.tile([P, QC], F32, name="nmsg")
                    nc.scalar.activation(nm[:], ms_b[:], AF.Identity,
                                         scale=ng1_sb[:, kc:kc + 1],
                                         bias=be1_sb[:, kc:kc + 1])
                    t = sqp.tile([P, QC], F32, name="t_ln")
                    nc.vector.scalar_tensor_tensor(
                        out=t[:], in0=y1_sb[:, kc].bitcast(F32),
                        scalar=g1_sb[:, kc:kc + 1], in1=rstd_b[:],
                        op0=OP.mult, op1=OP.mult)
                    nc.vector.tensor_add(z1_sb[:, kc].bitcast(F32), t[:],
                                         nm[:])
                    jh, ih = kc // 2, kc % 2
                    nc.scalar.activation(z1h_sb[:, jh, ih, :],
                                         z1_sb[:, kc].bitcast(F32), AF.Copy)
                    nc.vector.tensor_sub(z1l_sb[:, jh, ih, :],
                                         z1_sb[:, kc].bitcast(F32),
                                         z1h_sb[:, jh, ih, :])
                    # fold the FFN output bias into z1 (its only remaining
                    # consumer is y2 = f + b2 + z1) on the scalar engine
                    nc.scalar.activation(z1_sb[:, kc].bitcast(F32),
                                         z1_sb[:, kc].bitcast(F32),
                                         AF.Identity,
                                         bias=b2t_sb[:, kc:kc + 1])

                ln_finish(st1, z1_emit, "ln1")

            # ---- FFN1 + FFN2(first half) interleaved. fp8 DR with hi/lo
            # error compensation: keep hi@hi + lo@hi + hi@lo. Matmuls are
            # emitted per-output-tile (12 back-to-back accumulations) so
            # each h_ps tile finishes early and its relu overlaps the next
            # tile's matmuls — h_ps stays single-buffered (8 PSUM banks
            # total: 4 h_ps + 4 f_ps). ----
            def y2_emit(m, fp):
                # f_ps = 32*f; y2 = f + (z1 + b2)  (b2 folded into z1)
                nc.vector.scalar_tensor_tensor(
                    out=y2_sb[:, m], in0=fp[:],
                    scalar=1.0 / 32.0, in1=z1_sb[:, m].bitcast(F32),
                    op0=OP.mult, op1=OP.add)

            with tc.tile_pool(name="psf", bufs=1, space="PSUM") as psf:
                f_ps = [psf.tile([P, QC], F32, name=f"f_ps{i}")
                        for i in range(4)]
                for fg in range(KF // 4):
                    w1h_t = wp.tile([P, KD2, 2, 512], F8, name="w1h_t")
                    w1l_t = wp.tile([P, KD2, 2, 512], F8, name="w1l_t")
                    nc.sync.dma_start(w1h_t[:],
                                      w1h8[:, :, :, fg * 512:(fg + 1) * 512])
                    nc.sync.dma_start(w1l_t[:],
                                      w1l8[:, :, :, fg * 512:(fg + 1) * 512])
                    h_ps = [psf.tile([P, QC], F32, name=f"h_ps{i}")
                            for i in range(4)]
                    hb = hbp.tile([P, 4, QC], BF16, name="hb")
                    for i in range(4):
                        # per-tile-major: each tile finishes early so its
                        # relu overlaps the next tile's matmuls
                        for pi, (wt, zt) in enumerate(((w1h_t, z1h_sb),
                                                       (w1l_t, z1h_sb),
                                                       (w1h_t, z1l_sb))):
                            for j in range(KD2):
                                nc.tensor.matmul(
                                    h_ps[i][:],
                                    wt[:, j, :, i * P:(i + 1) * P],
                                    zt[:, j], start=(j == 0 and pi == 0),
                                    stop=(j == KD2 - 1 and pi == 2),
                                    perf_mode=DR)
                        fm = fg * 4 + i
                        # h_ps = 16*(z1@W1^T); relu(x/16 + b1). The hi fp8
                        # copy is a second relu on the scalar engine; the lo
                        # residual runs on Pool — both off the DVE.
                        jp, ip = fm // 2, fm % 2
                        nc.scalar.activation(hb[:, i, :], h_ps[i][:], AF.Relu,
                                             bias=b1t_sb[:, fm:fm + 1],
                                             scale=1.0 / 16.0)
                        nc.scalar.activation(hh_sb[:, jp, ip, :], h_ps[i][:],
                                             AF.Relu,
                                             bias=b1t_sb[:, fm:fm + 1],
                                             scale=1.0 / 16.0)
                        nc.vector.tensor_sub(hl_sb[:, jp, ip, :],
                                             hb[:, i, :],
                                             hh_sb[:, jp, ip, :])
                    # FFN2 first output half: consume h pairs as they finish;
                    # the weight tiles prefetch one fg ahead.
                    if fg % 2 == 0:
                        g4p = fg // 2
                        w2h_t = wp.tile([P, 4, 2, 512], F8, name="w2h_t")
                        w2l_t = wp.tile([P, 4, 2, 512], F8, name="w2l_t")
                        nc.sync.dma_start(
                            w2h_t[:], w2h8[:, g4p * 4:(g4p + 1) * 4, :, 0:512])
                        nc.sync.dma_start(
                            w2l_t[:], w2l8[:, g4p * 4:(g4p + 1) * 4, :, 0:512])
                    if fg % 2 == 1:
                        g4 = fg // 2       # group of 4 jp pairs
                        for i in range(4):
                            for jj in range(4):
                                jp = g4 * 4 + jj
                                for wt, ht, pi in ((w2h_t, hh_sb, 0),
                                                   (w2l_t, hh_sb, 1),
                                                   (w2h_t, hl_sb, 2)):
                                    nc.tensor.matmul(
                                        f_ps[i][:],
                                        wt[:, jj, :, i * P:(i + 1) * P],
                                        ht[:, jp], start=(jp == 0 and pi == 0),
                                        stop=(jp == KF2 - 1 and pi == 2),
                                        perf_mode=DR)
                for i in range(4):
                    y2_emit(i, f_ps[i])

            # FFN2 second output half (fresh PSUM pool after psf closes)
            with tc.tile_pool(name="psf2", bufs=1, space="PSUM") as psf2, \
                 tc.tile_pool(name="psst2", bufs=1, space="PSUM") as psst2:
                st2 = psst2.tile([1, 1024], F32, name="st2")

                def y2_stats(m):
                    _mm(nc, st2[:, 0:512], ones[:], y2_sb[:, m],
                        start=(m == 0), stop=(m == KD - 1))
                    sq = sqp.tile([P, QC], F32R, name="sq")
                    nc.scalar.activation(sq[:], y2_sb[:, m].bitcast(F32),
                                         AF.Square)
                    _mm(nc, st2[:, 512:1024], ones[:], sq[:],
                        start=(m == 0), stop=(m == KD - 1))

                # first-half y2 stats run under the mg1 matmuls
                for m in range(4):
                    y2_stats(m)
                f_ps2 = [psf2.tile([P, QC], F32, name=f"f2_ps{i}")
                         for i in range(4)]
                for g4 in range(4):
                    w2h_t = wp.tile([P, 4, 2, 512], F8, name="w2h_t")
                    w2l_t = wp.tile([P, 4, 2, 512], F8, name="w2l_t")
                    nc.sync.dma_start(
                        w2h_t[:], w2h8[:, g4 * 4:(g4 + 1) * 4, :, 512:1024])
                    nc.sync.dma_start(
                        w2l_t[:], w2l8[:, g4 * 4:(g4 + 1) * 4, :, 512:1024])
                    for i in range(4):
                        for jj in range(4):
                            jp = g4 * 4 + jj
                            for wt, ht, pi in ((w2h_t, hh_sb, 0),
                                               (w2l_t, hh_sb, 1),
                                               (w2h_t, hl_sb, 2)):
                                nc.tensor.matmul(
                                    f_ps2[i][:],
                                    wt[:, jj, :, i * P:(i + 1) * P],
                                    ht[:, jp], start=(jp == 0 and pi == 0),
                                    stop=(jp == KF2 - 1 and pi == 2),
                                    perf_mode=DR)
                for i in range(4):
                    y2_emit(4 + i, f_ps2[i])
                    y2_stats(4 + i)

                # ---- LN2 normalize -> z2 -> out. 2 DVE ops per chunk;
                # the (be - ms*g) tensor precomputes on Pool. ----
                def z2_emit(kc, rstd_b, ms_b):
                    nm = bcp.tile([P, QC], F32, name="nmsg")
                    nc.scalar.activation(nm[:], ms_b[:], AF.Identity,
                                         scale=ng2_sb[:, kc:kc + 1],
                                         bias=be2_sb[:, kc:kc + 1])
                    t = sqp.tile([P, QC], F32, name=f"t_ln{kc % 2}")
                    nc.vector.scalar_tensor_tensor(
                        out=t[:], in0=y2_sb[:, kc].bitcast(F32),
                        scalar=g2_sb[:, kc:kc + 1], in1=rstd_b[:],
                        op0=OP.mult, op1=OP.mult)
                    nc.vector.tensor_add(z2_sb[:, kc], t[:], nm[:])
                    nc.sync.dma_start(out_t[kc * P:(kc + 1) * P, :],
                                      z2_sb[:, kc])

                ln_finish(st2, z2_emit, "ln2")
    nc.compile()
    return nc


def _get(name, builder):
    if name not in _CACHE:
        _CACHE[name] = builder()
    return _CACHE[name]


def _dr_pack(a, scale=1.0):
    """[Din, M] f32 -> [P, Din//256, 2, M] fp8 DoubleRow layout."""
    a = np.asarray(a, np.float32) * scale
    din, m = a.shape
    return np.ascontiguousarray(
        a.reshape(din // 256, 2, P, m).transpose(2, 0, 1, 3).astype(NP_F8))


def _prep_inputs(X, Wq, Wk, Wo, ln1_g, ln1_b, ln2_g, ln2_b, W1, b1, W2, b2):
    f = lambda a: np.ascontiguousarray(np.asarray(a, dtype=np.float32))
    Xf = np.asarray(X, np.float32).reshape(N, D)
    Xt = f(Xf.T)                                              # [D, N]
    xt8 = _dr_pack(Xt)                                        # [P,4,2,N] fp8
    WqT, WkT, WoT = (np.asarray(w, np.float32).T for w in (Wq, Wk, Wo))
    vecP = lambda v, k: f(np.asarray(v).reshape(k, P).T)      # [P, k]
    parsv = np.concatenate(
        [vecP(ln1_g, D // P), vecP(ln1_b, D // P), vecP(ln2_g, D // P),
         vecP(ln2_b, D // P), vecP(b2, D // P), vecP(b1, FF // P)], axis=1)

    in_maps_a = [
        {
            "xt8": xt8,
            "wq8": _dr_pack(WqT[:, c * P:(c + 1) * P], 16.0),
            "wk8": _dr_pack(WkT[:, c * P:(c + 1) * P], 16.0),
            "wv8": _dr_pack(WoT[:, c * P:(c + 1) * P], 16.0),
        }
        for c in range(N_CORES)
    ]

    wot8 = _dr_pack(WoT, 16.0)
    W1sT = np.asarray(W1, np.float32).T * 16.0                # [D, FF]
    w1h = W1sT.astype(NP_F8)
    w1l = (W1sT - w1h.astype(np.float32)).astype(NP_F8)
    W2sT = np.asarray(W2, np.float32).T * 32.0                # [FF, D]
    w2h = W2sT.astype(NP_F8)
    w2l = (W2sT - w2h.astype(np.float32)).astype(NP_F8)
    pack8 = lambda a: np.ascontiguousarray(
        a.reshape(a.shape[0] // 256, 2, P, a.shape[1]).transpose(2, 0, 1, 3))
    w1h8, w1l8 = pack8(w1h), pack8(w1l)
    w2h8, w2l8 = pack8(w2h), pack8(w2l)

    def in_maps_b(ctx_full):
        # ctx_full: [N, D] f32 natural layout
        ct_t = ctx_full.T                                     # [D, N]
        return [
            {
                "ct8": _dr_pack(ct_t[:, c * QC:(c + 1) * QC], 16.0),
                "xts": f(Xt[:, c * QC:(c + 1) * QC]),
                "wot8": wot8,
                "w1h8": w1h8, "w1l8": w1l8,
                "w2h8": w2h8, "w2l8": w2l8,
                "pars": parsv,
            }
            for c in range(N_CORES)
        ]

    return in_maps_a, in_maps_b


def kernel(X, Wq, Wk, Wo, ln1_g, ln1_b, ln2_g, ln2_b, W1, b1, W2, b2):
    in_maps_a, in_maps_b = _prep_inputs(
        X, Wq, Wk, Wo, ln1_g, ln1_b, ln2_g, ln2_b, W1, b1, W2, b2)

    nc_a = _get("a", _build_phase_a)
    res_a = run_bass_kernel_spmd(nc_a, in_maps_a, core_ids=list(range(N_CORES)))
    # ctx_n[c] is [N, 128]: head-block columns for heads (2c, 2c+1)
    ctx_full = np.concatenate(
        [res_a.results[c]["ctx_n"] for c in range(N_CORES)], axis=1)  # [N, D]

    nc_b = _get("b", _build_phase_b)
    res_b = run_bass_kernel_spmd(nc_b, in_maps_b(ctx_full),
                                 core_ids=list(range(N_CORES)))
    out_t = np.concatenate(
        [res_b.results[c]["out_t"] for c in range(N_CORES)], axis=1)  # [D, N]
    return np.ascontiguousarray(out_t.T).reshape(B, S, D).astype(np.float32)


# revision 45
# speedup vs baseline: 1.2670x; 1.0105x over previous
"""Trainium2 Bass kernel for nn_Encoder (dense transformer block), 8 NeuronCores.

Strategy (single chip, 8 cores), v2 with fp8 DoubleRow matmuls:
  Phase A (head-parallel): core c computes attention for heads {2c, 2c+1}.
    Q/K projections run as fp8e4 DoubleRow matmuls (256-deep contraction per
    instruction at 0.5 cyc/row) off an SBUF-resident fp8 X^T; V is projected
    directly into natural [token, dim] layout (also fp8-DR), removing the PE
    transposes. Scores stay fp32r; softmax(relu(s)) = max(exp(s), 1) with a
    bf16 slab, and ctx accumulates in natural layout ([128-token, 65] tiles,
    bf16 at 65 cyc/instr) so the row-sum normalize is a per-partition scalar
    multiply. Output is natural-layout f32 ctx.
  Phase B (row-parallel): core c takes 512 of the 4096 token rows.
    Wo-projection runs fp8-DR off a x16-scaled fp8 ctx; the FFN runs
    error-compensated fp8-DR (x = hi + lo split of both activations and
    weights, keeping the hi@hi + lo@hi + hi@lo terms: 0.75 cyc/row at
    ~bf16 accuracy). LayerNorm reductions run over the partition dim via
    ones-vector matmuls on the PE as before.

kernel() is self-contained: it compiles both phase programs on first call
(cached in module globals) and runs them via run_bass_kernel_spmd.
"""

import os
import sys

for _p in ("/opt/trn_rl_repo",):
    if _p not in sys.path:
        sys.path.insert(0, _p)

# The Bass SPMD path executes through jax/PJRT on the axon platform; make
# sure a caller-pinned JAX_PLATFORMS=cpu doesn't hide the NeuronCores.
_jp = os.environ.get("JAX_PLATFORMS")
if _jp is not None and "axon" not in _jp:
    os.environ["JAX_PLATFORMS"] = "axon," + _jp

import ml_dtypes
import numpy as np

import concourse.bass as bass
import concourse.mybir as mybir
import concourse.tile as tile
from concourse import bacc
from concourse.bass_utils import run_bass_kernel_spmd

F32 = mybir.dt.float32
F32R = mybir.dt.float32r
BF16 = mybir.dt.bfloat16
F8 = mybir.dt.float8e4
AF = mybir.ActivationFunctionType
OP = mybir.AluOpType
DR = mybir.MatmulPerfMode.DoubleRow

NP_F8 = ml_dtypes.float8_e4m3

N_CORES = 8
B, S, D, H, DH, FF = 2, 2048, 1024, 16, 64, 4096
N = B * S            # 4096 token rows
P = 128
QC = N // N_CORES    # 512 rows per core in phase B
KD2 = D // 256       # 4 DoubleRow contraction pairs over D
KF2 = FF // 256      # 16 DoubleRow contraction pairs over FF
KI = S // P          # 16 key chunks of 128 per batch
EPS = 1e-5

_CACHE = {}


def _mm(nc, out, lhsT, rhs, **kw):
    # fp32r: 1-pass FP22 matmul
    nc.tensor.matmul(out, lhsT.bitcast(F32R), rhs.bitcast(F32R), **kw)


# --------------------------------------------------------------------------
# Phase A: per-core head-parallel attention.
# Inputs (per core):
#   xt8  [P, KD2, 2, N]  fp8 X^T in DoubleRow layout: [p,j,i,n] = X[n, d],
#                        d = (2j+i)*128 + p
#   wq8/wk8/wv8 [P, KD2, 2, P]  fp8 16*W{q,k,o}^T columns for this core's two
#                        heads, same d-mapping on the partition side
# Output:
#   ctx_n [N, P] f32  natural-layout ctx: row n, cols 0:64 head0, 64:128 head1
# --------------------------------------------------------------------------
def _build_phase_a():
    nc = bacc.Bacc("TRN2", target_bir_lowering=False, debug=False,
                   num_devices=N_CORES)
    xt8 = nc.dram_tensor("xt8", [P, KD2, 2, N], F8, kind="ExternalInput")
    wq8 = nc.dram_tensor("wq8", [P, KD2, 2, P], F8, kind="ExternalInput")
    wk8 = nc.dram_tensor("wk8", [P, KD2, 2, P], F8, kind="ExternalInput")
    wv8 = nc.dram_tensor("wv8", [P, KD2, 2, P], F8, kind="ExternalInput")
    ctx_n = nc.dram_tensor("ctx_n", [N, P], F32, kind="ExternalOutput")

    NO = S // 512      # 4 query slices of 512 per batch

    with tile.TileContext(nc) as tc:
        with tc.tile_pool(name="persist", bufs=1) as persist:
            xt_sb = persist.tile([P, KD2, 2, N], F8)
            wq_sb = persist.tile([P, KD2, 2, P], F8)
            wk_sb = persist.tile([P, KD2, 2, P], F8)
            wv_sb = persist.tile([P, KD2, 2, P], F8)
            qt_sb = [persist.tile([P, S], F32R, name=f"qt{b_}") for b_ in range(B)]
            kt_sb = [persist.tile([P, S], F32R, name=f"kt{b_}") for b_ in range(B)]
            # V' natural layout per ki chunk: [v_h0(64) | 1 | v_h1(64) | 1]
            vp_sb = [persist.tile([P, KI, 130], BF16, name=f"vp{b_}")
                     for b_ in range(B)]

            # HWDGE descriptor generation serializes DMAs (~625ns each), so
            # order by first use: X^T chunk 0, the q/k weights, then the rest.
            nc.sync.dma_start(xt_sb[:, :, :, 0:512], xt8[:, :, :, 0:512])
            nc.sync.dma_start(wq_sb[:], wq8.ap())
            nc.sync.dma_start(wk_sb[:], wk8.ap())
            nc.sync.dma_start(wv_sb[:], wv8.ap())
            for o8 in range(1, 8):
                nc.sync.dma_start(xt_sb[:, :, :, o8 * 512:(o8 + 1) * 512],
                                  xt8[:, :, :, o8 * 512:(o8 + 1) * 512])
            for b_ in range(B):
                nc.vector.memset(vp_sb[b_][:, :, 64:65], 1.0)
                nc.vector.memset(vp_sb[b_][:, :, 129:130], 1.0)

            with (
                # shared PSUM ring: scores tiles, projection accumulators
                tc.tile_pool(name="ring", bufs=3, space="PSUM") as ring,
                tc.tile_pool(name="cpool", bufs=1, space="PSUM") as cpool,
                tc.tile_pool(name="slabp", bufs=26) as slabp,
                tc.tile_pool(name="coutp", bufs=3) as coutp,
                tc.tile_pool(name="rcp", bufs=4) as rcp,
            ):
                def proj_qk(b_, o):
                    """Project one 512-token slice of batch b_ into qt/kt
                    (t-layout, fp8 DoubleRow, values are 16*q / 16*k)."""
                    sl = slice(b_ * S + o * 512, b_ * S + (o + 1) * 512)
                    for w_sb, dst in ((wq_sb, qt_sb[b_]), (wk_sb, kt_sb[b_])):
                        acc = ring.tile([P, 2, 512], F32, name="rps")
                        for j in range(KD2):
                            nc.tensor.matmul(
                                acc[:, 0, :], w_sb[:, j], xt_sb[:, j, :, sl],
                                start=(j == 0), stop=(j == KD2 - 1),
                                perf_mode=DR)
                        nc.vector.tensor_copy(
                            dst[:, o * 512:(o + 1) * 512], acc[:, 0, :])

                def proj_v(b_, ki):
                    """Project one 128-token chunk of V directly into natural
                    layout (rows = tokens): vp[:, ki, 0:64]=16*v_h0,
                    [65:129]=16*v_h1."""
                    t0 = b_ * S + ki * P
                    acc = ring.tile([P, 2, 512], F32, name="rps")
                    for j in range(KD2):
                        nc.tensor.matmul(
                            acc[:, 0, 0:P], xt_sb[:, j, :, t0:t0 + P],
                            wv_sb[:, j], start=(j == 0), stop=(j == KD2 - 1),
                            perf_mode=DR)
                    for hh in range(2):
                        nc.vector.tensor_copy(
                            vp_sb[b_][:, ki, hh * 65:hh * 65 + 64],
                            acc[:, 0, hh * 64:(hh + 1) * 64])

                chunks = [(b_, o) for b_ in range(B) for o in range(NO)]
                slabs = {}
                cnat = {}

                def unit(idx, pr, h, split=False):
                    """Scores + exp + max for (query chunk idx, key pair pr,
                    head h): 2x[128,512] fp32r scores, one exp over 1024.
                    split=True emits per-query-half score/exp pairs (same
                    values) so the first exp starts earlier at startup."""
                    b_, o = chunks[idx]
                    hs = slice(h * DH, (h + 1) * DH)
                    sp = ring.tile([P, 2, 512], F32, name="rps")
                    slab = slabp.tile([P, 2, 512], BF16, name="slab")
                    halves = ((0, 256), (256, 512)) if split else ((0, 512),)
                    for q0, q1 in halves:
                        qs = slice(o * 512 + q0, o * 512 + q1)
                        for i2 in range(2):
                            ks = (pr * 2 + i2) * P
                            _mm(nc, sp[:, i2, q0:q1], kt_sb[b_][hs, ks:ks + P],
                                qt_sb[b_][hs, qs], start=True, stop=True)
                        # scores carry 16*16=256 scaling; softmax scale 1/8
                        nc.scalar.activation(slab[:, :, q0:q1],
                                             sp[:, :, q0:q1], AF.Exp,
                                             scale=1.0 / 2048.0)
                        nc.vector.tensor_scalar_max(slab[:, :, q0:q1],
                                                    slab[:, :, q0:q1], 1.0)
                    slabs[(idx, pr, h)] = slab

                def ctx_slot(idx, t, h):
                    """ctx accumulation for one [128-token, 65] output slot.
                    The 32 matmuls of each slot run as one CONTIGUOUS
                    accumulation group: hardware PSUM accumulation breaks
                    when groups at different offsets of the same bank
                    interleave (one accumulation per interruption is lost)."""
                    b_, o = chunks[idx]
                    c = cnat[idx]
                    for pr in range(KI // 2):
                        slab = slabs[(idx, pr, h)]
                        for i2 in range(2):
                            ki = pr * 2 + i2
                            nc.tensor.matmul(
                                c[:, t, h, 0:65],
                                slab[:, i2, t * P:(t + 1) * P],
                                vp_sb[b_][:, ki, h * 65:(h + 1) * 65],
                                start=(pr == 0 and i2 == 0),
                                stop=(pr == KI // 2 - 1 and i2 == 1))
                    if t == 3:
                        for pr in range(KI // 2):
                            slabs.pop((idx, pr, h))

                def normalize(idx):
                    b_, o = chunks[idx]
                    c = cnat.pop(idx)
                    row0 = b_ * S + o * 512
                    # all reciprocals first: the dependent scalar-multiplies
                    # then pipeline without per-pair semaphore round-trips
                    rc = rcp.tile([P, 4, 2], F32, name="rc")
                    for t in range(4):
                        for h in range(2):
                            nc.vector.reciprocal(rc[:, t, h:h + 1],
                                                 c[:, t, h, 64:65])
                    for t in range(4):
                        cout = coutp.tile([P, 2, 64], F32, name="cout")
                        for h in range(2):
                            # ctx carries a x16 scale from 16*v
                            nc.vector.tensor_scalar(
                                out=cout[:, h, :], in0=c[:, t, h, 0:64],
                                scalar1=rc[:, t, h:h + 1], scalar2=1.0 / 16.0,
                                op0=OP.mult, op1=OP.mult)
                        nc.sync.dma_start(
                            ctx_n[row0 + t * P:row0 + (t + 1) * P, :],
                            cout[:])

                # ---- schedule: chunk idx-1's eight ctx slot-groups are
                # spread across chunk idx's 16 score units (h0 groups first,
                # then h1); batch-1 projection chains spread across the unit
                # slots so the scalar engine's exp stream never starves at
                # chunk boundaries.
                slots = [(pr, h) for pr in range(KI // 2) for h in range(2)]
                groups = [(t, h) for h in range(2) for t in range(4)]
                from collections import deque

                proj_qk(0, 0)
                for o in range(NO):
                    us = [(pr, h) for pr in (2 * o, 2 * o + 1)
                          for h in range(2)]
                    unit(0, *us[0])
                    proj_v(0, o * 4)
                    unit(0, *us[1])
                    proj_v(0, o * 4 + 1)
                    unit(0, *us[2])
                    if o + 1 < NO:
                        proj_qk(0, o + 1)
                    unit(0, *us[3])
                    proj_v(0, o * 4 + 2)
                    proj_v(0, o * 4 + 3)
                for idx in range(1, len(chunks)):
                    projs = deque()
                    if idx <= NO:
                        o = idx - 1
                        projs.append(lambda o=o: proj_qk(1, o))
                        for kk in range(4):
                            projs.append(
                                lambda k=o * 4 + kk: proj_v(1, k))
                    cnat[idx - 1] = cpool.tile([P, 4, 2, P], F32, name="cnat")
                    gsched = {1: 0, 3: 1, 5: 2, 7: 3, 9: 4, 11: 5,
                              13: 6, 14: 7}
                    for u in range(len(slots)):
                        if projs and u % 3 == 0:
                            projs.popleft()()
                        unit(idx, *slots[u])
                        if u in gsched:
                            ctx_slot(idx - 1, *groups[gsched[u]])
                            if gsched[u] == 7:
                                normalize(idx - 1)
                    while projs:
                        projs.popleft()()
                last = len(chunks) - 1
                cnat[last] = cpool.tile([P, 4, 2, P], F32, name="cnat")
                for t, h in groups:
                    ctx_slot(last, t, h)
                normalize(last)
    nc.compile()
    return nc


# --------------------------------------------------------------------------
# Phase B: per-core row-parallel Wo-proj + AddNorm1 + FFN + AddNorm2.
# Inputs (per core, qi = this core's 512 token rows):
#   ct8   [P, KD2, 2, QC] fp8  16*ctx^T slice, DoubleRow layout
#   xts   [D, QC] f32          X^T slice (residual 1)
#   wot8  [P, KD2, 2, D] fp8   16*Wo^T, DoubleRow layout
#   w1h8/w1l8 [P, KD2, 2, FF] fp8   hi/lo split of 16*W1^T
#   w2h8/w2l8 [P, KF2, 2, D]  fp8   hi/lo split of 32*W2^T
#   g1,be1,g2,be2 [P, D//P], b1t [P, FF//P], b2t [P, D//P]
# Output: out_t [D, QC] f32
# --------------------------------------------------------------------------
def _build_phase_b():
    nc = bacc.Bacc("TRN2", target_bir_lowering=False, debug=False,
                   num_devices=N_CORES)
    ct8 = nc.dram_tensor("ct8", [P, KD2, 2, QC], F8, kind="ExternalInput")
    xts = nc.dram_tensor("xts", [D, QC], F32, kind="ExternalInput")
    wot8 = nc.dram_tensor("wot8", [P, KD2, 2, D], F8, kind="ExternalInput")
    w1h8 = nc.dram_tensor("w1h8", [P, KD2, 2, FF], F8, kind="ExternalInput")
    w1l8 = nc.dram_tensor("w1l8", [P, KD2, 2, FF], F8, kind="ExternalInput")
    w2h8 = nc.dram_tensor("w2h8", [P, KF2, 2, D], F8, kind="ExternalInput")
    w2l8 = nc.dram_tensor("w2l8", [P, KF2, 2, D], F8, kind="ExternalInput")
    # g1 | be1 | g2 | be2 | b2t (8 cols each) | b1t (32 cols)
    pars = nc.dram_tensor("pars", [P, 72], F32, kind="ExternalInput")
    out_t = nc.dram_tensor("out_t", [D, QC], F32, kind="ExternalOutput")

    KD = D // P     # 8
    KF = FF // P    # 32

    with tile.TileContext(nc) as tc:
        with (
            tc.tile_pool(name="persist", bufs=1) as persist,
            tc.tile_pool(name="wp", bufs=2) as wp,
            tc.tile_pool(name="hbp", bufs=2) as hbp,
            tc.tile_pool(name="sqp", bufs=3) as sqp,
            tc.tile_pool(name="smallp", bufs=1) as smallp,
            tc.tile_pool(name="bcp", bufs=2) as bcp,
        ):
            ct_sb = persist.tile([P, KD2, 2, QC], F8)
            pars_sb = persist.tile([P, 72], F32)
            wot_sb = persist.tile([P, KD2, 2, D], F8)
            xts_sb = persist.tile([P, KD, QC], F32)
            y1_sb = persist.tile([P, KD, QC], F32R)
            z1_sb = persist.tile([P, KD, QC], F32R)
            z1h_sb = persist.tile([P, KD2, 2, QC], F8)
            z1l_sb = persist.tile([P, KD2, 2, QC], F8)
            hh_sb = persist.tile([P, KF2, 2, QC], F8)
            hl_sb = persist.tile([P, KF2, 2, QC], F8)
            # y2 reuses y1's slot (y1 dead after LN1); z2 reuses xts's (dead
            # after the y1 adds).
            y2_sb = persist.tile([P, KD, QC], F32R, tag="y1_sb")
            z2_sb = persist.tile([P, KD, QC], F32, tag="xts_sb")
            g1_sb = pars_sb[:, 0:8]
            be1_sb = pars_sb[:, 8:16]
            g2_sb = pars_sb[:, 16:24]
            be2_sb = pars_sb[:, 24:32]
            b2t_sb = pars_sb[:, 32:40]
            b1t_sb = pars_sb[:, 40:72]
            ones = persist.tile([P, 1], F32R)

            nc.sync.dma_start(ct_sb[:], ct8.ap())
            # split so the first Wo matmuls start after ~a quarter of the
            # weight transfer
            nc.sync.dma_start(wot_sb[:, :, :, 0:256], wot8[:, :, :, 0:256])
            nc.sync.dma_start(wot_sb[:, :, :, 256:512], wot8[:, :, :, 256:512])
            nc.sync.dma_start(pars_sb[:], pars.ap())
            nc.sync.dma_start(
                xts_sb[:, 0:4],
                xts[0:512, :].rearrange("(kc p) q -> p kc q", p=P))
            nc.sync.dma_start(wot_sb[:, :, :, 512:1024], wot8[:, :, :, 512:1024])
            nc.sync.dma_start(
                xts_sb[:, 4:8],
                xts[512:1024, :].rearrange("(kc p) q -> p kc q", p=P))
            # 1/D so the stats matmuls produce means directly
            nc.vector.memset(ones[:].bitcast(F32), 1.0 / D)
            eps_sb = persist.tile([1, 1], F32)
            nc.vector.memset(eps_sb[:], EPS)
            ng1_sb = persist.tile([P, KD], F32)
            ng2_sb = persist.tile([P, KD], F32)
            nc.vector.tensor_scalar(out=ng1_sb, in0=g1_sb, scalar1=-1.0,
                                    scalar2=None, op0=OP.mult)
            nc.vector.tensor_scalar(out=ng2_sb, in0=g2_sb, scalar1=-1.0,
                                    scalar2=None, op0=OP.mult)

            def ln_finish(st_ps, z_of_kc, tag):
                """Scalar chain of a layernorm. st_ps [1, 1024] holds the
                per-token mean and mean-square directly (the ones vector is
                1/D); EPS rides the sqrt's bias input. Calls
                z_of_kc(kc, rstd_b, ms_b) to emit per-chunk normalizes."""
                mean = st_ps[:, 0:512]
                ex2 = st_ps[:, 512:1024]
                var = smallp.tile([1, 512], F32, name=f"var_{tag}")
                # mean^2 on Act (the DVE can't read two PSUM operands)
                nc.scalar.activation(var[:], mean, AF.Square)
                nc.vector.tensor_sub(var[:], ex2, var[:])
                std = smallp.tile([1, 512], F32, name=f"std_{tag}")
                nc.scalar.activation(std[:], var[:], AF.Sqrt,
                                     bias=eps_sb[:])
                rstd = smallp.tile([1, 512], F32, name=f"rstd_{tag}")
                nc.vector.reciprocal(rstd[:], std[:])
                rstd_b = bcp.tile([P, 512], F32, name="rstd_b")
                nc.gpsimd.partition_broadcast(rstd_b[:], rstd[:])
                ms = smallp.tile([1, 512], F32, name=f"ms_{tag}")
                nc.vector.tensor_mul(ms[:], mean, rstd[:])
                ms_b = bcp.tile([P, 512], F32, name="ms_b")
                nc.gpsimd.partition_broadcast(ms_b[:], ms[:])
                for kc in range(KD):
                    z_of_kc(kc, rstd_b, ms_b)

            # ---- B1: att^T = (16 Wo^T)(16 ctx) via fp8 DR (+ X residual),
            # with LN1 sum/sumsq matmuls interleaved per finished chunk.
            # Matmuls are emitted per-output-tile so each a_ps buffer
            # completes early and its consumer never gates the next tile. ----
            with tc.tile_pool(name="psa", bufs=1, space="PSUM") as psa, \
                 tc.tile_pool(name="psst1", bufs=1, space="PSUM") as psst1:
                st1 = psst1.tile([1, 1024], F32, name="st1")
                for mg in range(2):
                    a_ps = [psa.tile([P, QC], F32, name=f"a_ps{i}")
                            for i in range(4)]
                    for i in range(4):
                        for j in range(KD2):
                            m0 = mg * 512 + i * P
                            nc.tensor.matmul(
                                a_ps[i][:], wot_sb[:, j, :, m0:m0 + P],
                                ct_sb[:, j], start=(j == 0),
                                stop=(j == KD2 - 1), perf_mode=DR)
                    for i in range(4):
                        m = mg * 4 + i
                        nc.vector.scalar_tensor_tensor(
                            out=y1_sb[:, m], in0=a_ps[i][:],
                            scalar=1.0 / 256.0, in1=xts_sb[:, m],
                            op0=OP.mult, op1=OP.add)
                        _mm(nc, st1[:, 0:512], ones[:], y1_sb[:, m],
                            start=(m == 0), stop=(m == KD - 1))
                        sq = sqp.tile([P, QC], F32R, name="sq")
                        # square on the (otherwise idle) scalar engine
                        nc.scalar.activation(sq[:], y1_sb[:, m].bitcast(F32),
                                             AF.Square)
                        _mm(nc, st1[:, 512:1024], ones[:], sq[:],
                            start=(m == 0), stop=(m == KD - 1))

                # ---- LN1 -> z1 (f32) + hi/lo fp8 split.
                # z = (y*g)*rstd + (be - ms*g): the tensor term precomputes
                # on Pool, the hi cast runs on Act — the DVE does 2 big ops
                # plus the lo residual per chunk. ----
                def z1_emit(kc, rstd_b, ms_b):
                    # nm = be - ms*g on the scalar engine (idle here)
                    nm = bcp.tile([P, QC], F32, name="nmsg")
                    nc.scalar.activation(nm[:], ms_b[:], AF.Identity,
                                         scale=ng1_sb[:, kc:kc + 1],
                                         bias=be1_sb[:, kc:kc + 1])
                    t = sqp.tile([P, QC], F32, name="t_ln")
                    nc.vector.scalar_tensor_tensor(
                        out=t[:], in0=y1_sb[:, kc].bitcast(F32),
                        scalar=g1_sb[:, kc:kc + 1], in1=rstd_b[:],
                        op0=OP.mult, op1=OP.mult)
                    nc.vector.tensor_add(z1_sb[:, kc].bitcast(F32), t[:],
                                         nm[:])
                    jh, ih = kc // 2, kc % 2
                    nc.scalar.activation(z1h_sb[:, jh, ih, :],
                                         z1_sb[:, kc].bitcast(F32), AF.Copy)

                ln_finish(st1, z1_emit, "ln1")
                for kc in range(KD):
                    jh, ih = kc // 2, kc % 2
                    nc.vector.tensor_sub(z1l_sb[:, jh, ih, :],
                                         z1_sb[:, kc].bitcast(F32),
                                         z1h_sb[:, jh, ih, :])
                    # fold the FFN output bias into z1 (its only remaining
                    # consumer is y2 = f + b2 + z1) on the scalar engine
                    nc.scalar.activation(z1_sb[:, kc].bitcast(F32),
                                         z1_sb[:, kc].bitcast(F32),
                                         AF.Identity,
                                         bias=b2t_sb[:, kc:kc + 1])

            # ---- FFN1 + FFN2(first half) interleaved. fp8 DR with hi/lo
            # error compensation: keep hi@hi + lo@hi + hi@lo. Matmuls are
            # emitted per-output-tile (12 back-to-back accumulations) so
            # each h_ps tile finishes early and its relu overlaps the next
            # tile's matmuls — h_ps stays single-buffered (8 PSUM banks
            # total: 4 h_ps + 4 f_ps). ----
            def y2_emit(m, fp):
                # f_ps = 32*f; y2 = f + (z1 + b2)  (b2 folded into z1)
                nc.vector.scalar_tensor_tensor(
                    out=y2_sb[:, m], in0=fp[:],
                    scalar=1.0 / 32.0, in1=z1_sb[:, m].bitcast(F32),
                    op0=OP.mult, op1=OP.add)

            with tc.tile_pool(name="psf", bufs=1, space="PSUM") as psf:
                f_ps = [psf.tile([P, QC], F32, name=f"f_ps{i}")
                        for i in range(4)]
                for fg in range(KF // 4):
                    w1h_t = wp.tile([P, KD2, 2, 512], F8, name="w1h_t")
                    w1l_t = wp.tile([P, KD2, 2, 512], F8, name="w1l_t")
                    nc.sync.dma_start(w1h_t[:],
                                      w1h8[:, :, :, fg * 512:(fg + 1) * 512])
                    nc.sync.dma_start(w1l_t[:],
                                      w1l8[:, :, :, fg * 512:(fg + 1) * 512])
                    h_ps = [psf.tile([P, QC], F32, name=f"h_ps{i}")
                            for i in range(4)]
                    hb = hbp.tile([P, 4, QC], BF16, name="hb")
                    for i in range(4):
                        # per-tile-major: each tile finishes early so its
                        # relu overlaps the next tile's matmuls
                        for pi, (wt, zt) in enumerate(((w1h_t, z1h_sb),
                                                       (w1l_t, z1h_sb),
                                                       (w1h_t, z1l_sb))):
                            for j in range(KD2):
                                nc.tensor.matmul(
                                    h_ps[i][:],
                                    wt[:, j, :, i * P:(i + 1) * P],
                                    zt[:, j], start=(j == 0 and pi == 0),
                                    stop=(j == KD2 - 1 and pi == 2),
                                    perf_mode=DR)
                        fm = fg * 4 + i
                        # h_ps = 16*(z1@W1^T); relu(x/16 + b1). The hi fp8
                        # copy is a second relu on the scalar engine; the lo
                        # residual runs on Pool — both off the DVE.
                        jp, ip = fm // 2, fm % 2
                        nc.scalar.activation(hb[:, i, :], h_ps[i][:], AF.Relu,
                                             bias=b1t_sb[:, fm:fm + 1],
                                             scale=1.0 / 16.0)
                        nc.scalar.activation(hh_sb[:, jp, ip, :], h_ps[i][:],
                                             AF.Relu,
                                             bias=b1t_sb[:, fm:fm + 1],
                                             scale=1.0 / 16.0)
                        nc.vector.tensor_sub(hl_sb[:, jp, ip, :],
                                             hb[:, i, :],
                                             hh_sb[:, jp, ip, :])
                    # FFN2 first output half: consume h pairs as they finish;
                    # the weight tiles prefetch one fg ahead.
                    if fg % 2 == 0:
                        g4p = fg // 2
                        w2h_t = wp.tile([P, 4, 2, 512], F8, name="w2h_t")
                        w2l_t = wp.tile([P, 4, 2, 512], F8, name="w2l_t")
                        nc.sync.dma_start(
                            w2h_t[:], w2h8[:, g4p * 4:(g4p + 1) * 4, :, 0:512])
                        nc.sync.dma_start(
                            w2l_t[:], w2l8[:, g4p * 4:(g4p + 1) * 4, :, 0:512])
                    if fg % 2 == 1:
                        g4 = fg // 2       # group of 4 jp pairs
                        for i in range(4):
                            for jj in range(4):
                                jp = g4 * 4 + jj
                                for wt, ht, pi in ((w2h_t, hh_sb, 0),
                                                   (w2l_t, hh_sb, 1),
                                                   (w2h_t, hl_sb, 2)):
                                    nc.tensor.matmul(
                                        f_ps[i][:],
                                        wt[:, jj, :, i * P:(i + 1) * P],
                                        ht[:, jp], start=(jp == 0 and pi == 0),
                                        stop=(jp == KF2 - 1 and pi == 2),
                                        perf_mode=DR)
                for i in range(4):
                    y2_emit(i, f_ps[i])

            # FFN2 second output half (fresh PSUM pool after psf closes)
            with tc.tile_pool(name="psf2", bufs=1, space="PSUM") as psf2, \
                 tc.tile_pool(name="psst2", bufs=1, space="PSUM") as psst2:
                st2 = psst2.tile([1, 1024], F32, name="st2")

                def y2_stats(m):
                    _mm(nc, st2[:, 0:512], ones[:], y2_sb[:, m],
                        start=(m == 0), stop=(m == KD - 1))
                    sq = sqp.tile([P, QC], F32R, name="sq")
                    nc.scalar.activation(sq[:], y2_sb[:, m].bitcast(F32),
                                         AF.Square)
                    _mm(nc, st2[:, 512:1024], ones[:], sq[:],
                        start=(m == 0), stop=(m == KD - 1))

                # first-half y2 stats run under the mg1 matmuls
                for m in range(4):
                    y2_stats(m)
                f_ps2 = [psf2.tile([P, QC], F32, name=f"f2_ps{i}")
                         for i in range(4)]
                for g4 in range(4):
                    w2h_t = wp.tile([P, 4, 2, 512], F8, name="w2h_t")
                    w2l_t = wp.tile([P, 4, 2, 512], F8, name="w2l_t")
                    nc.sync.dma_start(
                        w2h_t[:], w2h8[:, g4 * 4:(g4 + 1) * 4, :, 512:1024])
                    nc.sync.dma_start(
                        w2l_t[:], w2l8[:, g4 * 4:(g4 + 1) * 4, :, 512:1024])
                    for i in range(4):
                        for jj in range(4):
                            jp = g4 * 4 + jj
                            for wt, ht, pi in ((w2h_t, hh_sb, 0),
                                               (w2l_t, hh_sb, 1),
                                               (w2h_t, hl_sb, 2)):
                                nc.tensor.matmul(
                                    f_ps2[i][:],
                                    wt[:, jj, :, i * P:(i + 1) * P],
                                    ht[:, jp], start=(jp == 0 and pi == 0),
                                    stop=(jp == KF2 - 1 and pi == 2),
                                    perf_mode=DR)
                for i in range(4):
                    y2_emit(4 + i, f_ps2[i])
                    y2_stats(4 + i)

                # ---- LN2 normalize -> z2 -> out. 2 DVE ops per chunk;
                # the (be - ms*g) tensor precomputes on Pool. ----
                def z2_emit(kc, rstd_b, ms_b):
                    nm = bcp.tile([P, QC], F32, name="nmsg")
                    nc.scalar.activation(nm[:], ms_b[:], AF.Identity,
                                         scale=ng2_sb[:, kc:kc + 1],
                                         bias=be2_sb[:, kc:kc + 1])
                    t = sqp.tile([P, QC], F32, name=f"t_ln{kc % 2}")
                    nc.vector.scalar_tensor_tensor(
                        out=t[:], in0=y2_sb[:, kc].bitcast(F32),
                        scalar=g2_sb[:, kc:kc + 1], in1=rstd_b[:],
                        op0=OP.mult, op1=OP.mult)
                    nc.vector.tensor_add(z2_sb[:, kc], t[:], nm[:])
                    nc.sync.dma_start(out_t[kc * P:(kc + 1) * P, :],
                                      z2_sb[:, kc])

                ln_finish(st2, z2_emit, "ln2")
    nc.compile()
    return nc


def _get(name, builder):
    if name not in _CACHE:
        _CACHE[name] = builder()
    return _CACHE[name]


def _dr_pack(a, scale=1.0):
    """[Din, M] f32 -> [P, Din//256, 2, M] fp8 DoubleRow layout."""
    a = np.asarray(a, np.float32) * scale
    din, m = a.shape
    return np.ascontiguousarray(
        a.reshape(din // 256, 2, P, m).transpose(2, 0, 1, 3).astype(NP_F8))


def _prep_inputs(X, Wq, Wk, Wo, ln1_g, ln1_b, ln2_g, ln2_b, W1, b1, W2, b2):
    f = lambda a: np.ascontiguousarray(np.asarray(a, dtype=np.float32))
    Xf = np.asarray(X, np.float32).reshape(N, D)
    Xt = f(Xf.T)                                              # [D, N]
    xt8 = _dr_pack(Xt)                                        # [P,4,2,N] fp8
    WqT, WkT, WoT = (np.asarray(w, np.float32).T for w in (Wq, Wk, Wo))
    vecP = lambda v, k: f(np.asarray(v).reshape(k, P).T)      # [P, k]
    parsv = np.concatenate(
        [vecP(ln1_g, D // P), vecP(ln1_b, D // P), vecP(ln2_g, D // P),
         vecP(ln2_b, D // P), vecP(b2, D // P), vecP(b1, FF // P)], axis=1)

    in_maps_a = [
        {
            "xt8": xt8,
            "wq8": _dr_pack(WqT[:, c * P:(c + 1) * P], 16.0),
            "wk8": _dr_pack(WkT[:, c * P:(c + 1) * P], 16.0),
            "wv8": _dr_pack(WoT[:, c * P:(c + 1) * P], 16.0),
        }
        for c in range(N_CORES)
    ]

    wot8 = _dr_pack(WoT, 16.0)
    W1sT = np.asarray(W1, np.float32).T * 16.0                # [D, FF]
    w1h = W1sT.astype(NP_F8)
    w1l = (W1sT - w1h.astype(np.float32)).astype(NP_F8)
    W2sT = np.asarray(W2, np.float32).T * 32.0                # [FF, D]
    w2h = W2sT.astype(NP_F8)
    w2l = (W2sT - w2h.astype(np.float32)).astype(NP_F8)
    pack8 = lambda a: np.ascontiguousarray(
        a.reshape(a.shape[0] // 256, 2, P, a.shape[1]).transpose(2, 0, 1, 3))
    w1h8, w1l8 = pack8(w1h), pack8(w1l)
    w2h8, w2l8 = pack8(w2h), pack8(w2l)

    def in_maps_b(ctx_full):
        # ctx_full: [N, D] f32 natural layout
        ct_t = ctx_full.T                                     # [D, N]
        return [
            {
                "ct8": _dr_pack(ct_t[:, c * QC:(c + 1) * QC], 16.0),
                "xts": f(Xt[:, c * QC:(c + 1) * QC]),
                "wot8": wot8,
                "w1h8": w1h8, "w1l8": w1l8,
                "w2h8": w2h8, "w2l8": w2l8,
                "pars": parsv,
            }
            for c in range(N_CORES)
        ]

    return in_maps_a, in_maps_b


def kernel(X, Wq, Wk, Wo, ln1_g, ln1_b, ln2_g, ln2_b, W1, b1, W2, b2):
    in_maps_a, in_maps_b = _prep_inputs(
        X, Wq, Wk, Wo, ln1_g, ln1_b, ln2_g, ln2_b, W1, b1, W2, b2)

    nc_a = _get("a", _build_phase_a)
    res_a = run_bass_kernel_spmd(nc_a, in_maps_a, core_ids=list(range(N_CORES)))
    # ctx_n[c] is [N, 128]: head-block columns for heads (2c, 2c+1)
    ctx_full = np.concatenate(
        [res_a.results[c]["ctx_n"] for c in range(N_CORES)], axis=1)  # [N, D]

    nc_b = _get("b", _build_phase_b)
    res_b = run_bass_kernel_spmd(nc_b, in_maps_b(ctx_full),
                                 core_ids=list(range(N_CORES)))
    out_t = np.concatenate(
        [res_b.results[c]["out_t"] for c in range(N_CORES)], axis=1)  # [D, N]
    return np.ascontiguousarray(out_t.T).reshape(B, S, D).astype(np.float32)
